# revision 1
# baseline (speedup 1.0000x reference)
"""Trainium2 Bass kernel for nn_ANPToolEncoder (sparse attention encoder).

Sharding: M=64 context groups split across 8 NeuronCores (8 groups each);
the whole network is embarrassingly parallel in M — each core computes
out[:, m_shard, :] and the host concatenates. No collectives.

Layout convention on-chip: activations are kept "feat-major" ([feature
partitions, token free]) so every matmul contraction runs over the
partition axis; softmax denominators for self-attention are computed with
ones-vector matmuls on the PE; the cross-attention softmax denominator is
never computed at all (the final LayerNorm is invariant to per-row scale).
LayerNorm-then-linear (context LN -> V projection) is folded into the V
matmul via host-side weight scaling (wv*g) plus per-token istd/mu
correction terms.
"""

import os
import sys
import numpy as np

for _p in ("/opt/trn_rl_repo", "/root/.axon_site/_ro/trn_rl_repo"):
    if os.path.isdir(_p) and _p not in sys.path:
        sys.path.append(_p)

from concourse import bass, bacc, tile, mybir  # noqa: E402
from concourse.bass_utils import run_bass_kernel_spmd  # noqa: E402

# Pin the ACT function-table chooser to the one set that contains every
# function this kernel uses (exp/ln/relu/square/copy/identity). The default
# greedy chooser ping-pongs between exp_and_others and
# natural_log_exp_and_others, paying a ~1.3us table load dozens of times.
_ACT_PIN = "natural_log_exp_and_others"
_orig_get_act_tables = bacc.get_activation_tables


def _pinned_act_tables(arch):
    t = _orig_get_act_tables(arch)
    return {name: (fns if name == _ACT_PIN else set())
            for name, fns in t.items()}


bacc.get_activation_tables = _pinned_act_tables

B, M, C, DX, H, NH = 256, 64, 256, 512, 512, 8
DH = H // NH
EPS = 1e-5
N_CORES = 8
MLOC = M // N_CORES          # 8 context groups per core
KC = H // 128                # 4 feature chunks of 128
CC = C // 128                # 2 token chunks per group
BC = B // 128                # 2 query chunks

F32 = mybir.dt.float32
BF16 = mybir.dt.bfloat16
ACT = mybir.ActivationFunctionType
ALU = mybir.AluOpType

# compute dtype for matmul-feeding SBUF tensors ("f32" or "bf16")
COMPUTE_DT = os.environ.get("KERNEL_DT", "bf16")


def _np_dt(dt_str):
    if dt_str == "bf16":
        import ml_dtypes
        return np.dtype(ml_dtypes.bfloat16)
    return np.dtype(np.float32)


def _prep(inp):
    """Host-side weight folds + feat-major layouts. Returns (shared_map, per-core key)."""
    f = {k: np.asarray(v, np.float32) for k, v in inp.items()}
    w1 = f["cp_w1"]                                   # [H, DX+2]
    inw, inb = f["in_w"], f["in_b"]
    bq, bk, bv = inb[:H], inb[H:2*H], inb[2*H:]

    def chunkT(w):      # [out, in] -> [128, in/128, out]  (feat-major, k-chunked)
        wT = w.T                                      # [in, out]
        return np.ascontiguousarray(
            wT.reshape(wT.shape[0] // 128, 128, wT.shape[1]).transpose(1, 0, 2))

    def chunkv(v):      # [H] -> [128, KC]
        return np.ascontiguousarray(v.reshape(KC, 128).T)

    sH = 1.0 / np.sqrt(H)
    shared = {
        "w1kT": chunkT(w1[:, :DX]),                   # [128,4,512]
        "w1gT": np.ascontiguousarray(w1[:, DX:DX+2].T),  # [2,512]
        "b1":   chunkv(f["cp_b1"]),
        "w2T":  chunkT(f["cp_w2"]),
        "b2":   chunkv(f["cp_b2"]),
        "inwT": chunkT(inw),                          # [128,4,1536]
        "bq8":  chunkv(bq / np.sqrt(DH)),
        "bk":   chunkv(bk),
        "outwT": chunkT(f["out_w"]),
        "outbrow": (f["out_b"] + f["out_w"] @ bv)[None, :],   # [1,512]
        "wvgT": chunkT(f["wv_w"] * f["lnc_g"][None, :]),
        "kgb":  np.broadcast_to(f["wv_w"] @ f["lnc_g"], (128, H)).copy(),
        "kv2b": np.broadcast_to(f["wv_w"] @ f["lnc_b"] + f["wv_b"], (128, H)).copy(),
        # cross-attn K-proj folded away: logits_T = img @ W2', with
        # W2' = (sH*wk^T wq) @ qe^T + (sH*wk^T wq_b) broadcast; wk_b only
        # scales P~ columns, which the final LN's row-scale invariance eats
        "wkqT": chunkT(sH * f["wk_w"].T @ f["wq_w"]),
        "c2":   chunkv(sH * f["wk_w"].T @ f["wq_b"]),
        "lnogb": np.broadcast_to(f["lno_g"], (128, H)).copy(),
        "lnobb": np.broadcast_to(f["lno_b"], (128, H)).copy(),
        # query embed feat-major: [128, 4, 256]
        "qet": np.ascontiguousarray(
            f["query_embed"].T.reshape(KC, 128, B).transpose(1, 0, 2)),
    }

    # per-core X feat-major [128, 5, MLOC*C]: chunks 0-3 img dims, chunk 4
    # rows 0/1 = gt/pred (rest zero, never read)
    img, gt, pr = f["ctx_img_feat"], f["ctx_gt"], f["ctx_pred"]
    xts = []
    for ci in range(N_CORES):
        gs = slice(ci * MLOC, (ci + 1) * MLOC)
        xi = img[gs].reshape(MLOC * C, DX).T          # [512, 2048]
        xt = np.zeros((128, 5, MLOC * C), np.float32)
        xt[:, :4, :] = xi.reshape(4, 128, MLOC * C).transpose(1, 0, 2)
        xt[0, 4, :] = gt[gs].reshape(-1)
        xt[1, 4, :] = pr[gs].reshape(-1)
        xts.append(xt)
    return shared, xts


# names of DT-typed (matmul-feeding) params; everything else stays f32
_DT_PARAMS = {"w1kT", "w1gT", "w2T", "inwT", "outwT", "outbrow", "wvgT",
              "kgb", "kv2b", "wkqT", "lnogb", "lnobb", "qet", "xt"}

_SHAPES = {
    "xt":    [128, 5, MLOC * C],
    "qet":   [128, KC, B],
    "w1kT":  [128, KC, H], "w1gT": [2, H], "b1": [128, KC],
    "w2T":   [128, KC, H], "b2": [128, KC],
    "inwT":  [128, KC, 3 * H], "bq8": [128, KC], "bk": [128, KC],
    "outwT": [128, KC, H], "outbrow": [1, H],
    "wvgT":  [128, KC, H], "kgb": [128, H], "kv2b": [128, H],
    "wkqT":  [128, KC, H], "c2": [128, KC],
    "lnogb": [128, H], "lnobb": [128, H],
}


def _build(dt_str, skip_kv2, skip_lnog, skip_lnob, stage=99):
    DT = BF16 if dt_str == "bf16" else F32
    nc = bacc.Bacc("TRN2", target_bir_lowering=False, debug=False,
                   num_devices=N_CORES)

    P = {}
    for name, shp in _SHAPES.items():
        pdt = DT if name in _DT_PARAMS else F32
        P[name] = nc.declare_dram_parameter(name, shp, pdt, isOutput=False)
    out_ext = nc.declare_dram_parameter("out", [B, MLOC, H], F32, isOutput=True)

    expb = int(os.environ.get("KERNEL_EXPB", "1"))
    ps_bufs = 4 if expb else 7
    with tile.TileContext(nc) as tc:
        with tc.tile_pool(name="wt", bufs=1) as wt, \
             tc.tile_pool(name="wk", bufs=2) as wk, \
             tc.tile_pool(name="sm", bufs=3) as sm, \
             tc.tile_pool(name="ps", bufs=ps_bufs, space="PSUM") as psp, \
             tc.tile_pool(name="sc", bufs=(2 if expb else 1), space="PSUM") as scp, \
             tc.tile_pool(name="st", bufs=1, space="PSUM") as stp:

            # ---- load weights / consts ----
            unused = set()
            if skip_kv2:
                unused.add("kv2b")
            if skip_lnog:
                unused.add("lnogb")
            if skip_lnob:
                unused.add("lnobb")
            # spread input DMAs over several engine queues so the first
            # matmuls (Q proj, MLP1 of pair 0) aren't stuck behind one
            # serial queue; xt is sliced per pair so pair 0 lands first
            dma_eng = {
                "qet": nc.scalar, "wkqT": nc.scalar, "c2": nc.scalar,
                "w1kT": nc.scalar, "w1gT": nc.scalar, "b1": nc.scalar,
                "w2T": nc.gpsimd, "b2": nc.gpsimd, "inwT": nc.gpsimd,
                "outwT": nc.sync, "outbrow": nc.sync,
                "wvgT": nc.gpsimd, "kgb": nc.gpsimd, "kv2b": nc.gpsimd,
                "bq8": nc.scalar, "bk": nc.scalar,
                "lnogb": nc.sync, "lnobb": nc.sync,
            }
            dma_order = ["w1kT", "w1gT", "b1", "xt", "qet", "wkqT", "c2",
                         "w2T", "b2", "inwT", "bq8", "bk",
                         "outwT", "outbrow", "wvgT", "kgb", "kv2b",
                         "lnogb", "lnobb"]
            W = {}
            for name in dma_order:
                if name in unused:
                    continue
                pdt = DT if name in _DT_PARAMS else F32
                t = wt.tile(_SHAPES[name], pdt, tag=name)
                if name == "xt":
                    for gpd in range(MLOC // 2):
                        sl = slice(gpd * 2 * C, (gpd + 1) * 2 * C)
                        nc.sync.dma_start(out=t[:, :, sl], in_=P[name][:, :, sl])
                elif name == "w1kT":
                    for hcd in range(KC):
                        sl = slice(hcd * 128, (hcd + 1) * 128)
                        nc.scalar.dma_start(out=t[:, :, sl], in_=P[name][:, :, sl])
                else:
                    dma_eng[name].dma_start(out=t[...], in_=P[name][...])
                W[name] = t
            ones128 = wt.tile([128, 1], DT, tag="ones128")
            nc.vector.memset(ones128[:], 1.0)
            onesrow = wt.tile([1, 2 * C], DT, tag="onesrow")
            nc.vector.memset(onesrow[:], 1.0)
            onesbc = wt.tile([128, 64], F32, tag="onesbc")
            nc.vector.memset(onesbc[:], 1.0)
            epsc = wt.tile([128, 1], F32, tag="epsc")
            nc.vector.memset(epsc[:], EPS)

            def mm_chain(ps_ap, pairs):
                """Accumulating matmul chain: pairs = [(lhsT, rhs), ...]."""
                n = len(pairs)
                for i, (l, r) in enumerate(pairs):
                    nc.tensor.matmul(ps_ap, l, r, start=(i == 0), stop=(i == n - 1),
                                     skip_group_check=True)

            # ---- W2' = WKQ @ qe^T + c2 (once; replaces Q and K projs).
            # Emitted AFTER pair-0 projections: the in-order PE stream must
            # not start with matmuls that wait on late-arriving DMAs. ----
            W2T = wt.tile([128, KC, B], DT, tag="W2T")

            def emit_w2t():
                for dxc in range(KC):
                    ps = psp.tile([128, B], F32, tag="ps")
                    mm_chain(ps[...], [(W["wkqT"][:, k, dxc*128:(dxc+1)*128],
                                        W["qet"][:, k, :]) for k in range(KC)])
                    nc.scalar.activation(W2T[:, dxc, :], ps[...], ACT.Identity,
                                         bias=W["c2"][:, dxc:dxc+1])

            # ---- software-pipelined group-pair schedule ----
            # Projections for pair gp+1 are woven between the attention
            # stages of pair gp so the PE stream stays dense (HAM warm).

            def make_proj(gp):
                """Emitters for pair gp's batched (N=512) projections."""
                xg2 = slice(gp * 2 * C, (gp + 1) * 2 * C)
                PR = {}

                def p_mlp1():
                    h1 = wk.tile([128, KC, 2 * C], DT, tag="h1")
                    for hc in range(KC):
                        ps = psp.tile([128, 2 * C], F32, tag="ps")
                        pairs = [(W["w1kT"][:, k, hc*128:(hc+1)*128],
                                  W["xt"][:, k, xg2]) for k in range(4)]
                        pairs.append((W["w1gT"][0:2, hc*128:(hc+1)*128],
                                      W["xt"][0:2, 4, xg2]))
                        mm_chain(ps[...], pairs)
                        nc.scalar.activation(h1[:, hc, :], ps[...], ACT.Relu,
                                             bias=W["b1"][:, hc:hc+1])
                    PR["h1"] = h1

                def p_mlp2():
                    h1 = PR["h1"]
                    ctx = wk.tile([128, KC, 2 * C], DT, tag="ctx")
                    for hc in range(KC):
                        ps = psp.tile([128, 2 * C], F32, tag="ps")
                        mm_chain(ps[...], [(W["w2T"][:, k, hc*128:(hc+1)*128],
                                            h1[:, k, :]) for k in range(KC)])
                        nc.vector.tensor_scalar(ctx[:, hc, :], ps[...],
                                                W["b2"][:, hc:hc+1], None, ALU.add)
                    PR["ctx"] = ctx

                def p_qk():
                    ctx = PR["ctx"]
                    qk = wk.tile([128, 2 * KC, 2 * C], DT, tag="qk")
                    for jc in range(2 * KC):
                        ps = psp.tile([128, 2 * C], F32, tag="ps")
                        mm_chain(ps[...], [(W["inwT"][:, k, jc*128:(jc+1)*128],
                                            ctx[:, k, :]) for k in range(KC)])
                        if jc < KC:
                            nc.vector.tensor_scalar(qk[:, jc, :], ps[...],
                                                    1.0 / float(np.sqrt(DH)),
                                                    W["bq8"][:, jc:jc+1],
                                                    ALU.mult, ALU.add)
                        else:
                            nc.vector.tensor_scalar(qk[:, jc, :], ps[...],
                                                    W["bk"][:, jc-KC:jc-KC+1], None,
                                                    ALU.add)
                    PR["qk"] = qk

                def p_v():
                    ctx = PR["ctx"]
                    vtok = wk.tile([128, 2 * CC, H], DT, tag="vtok")
                    for cc2 in range(2 * CC):
                        ps = psp.tile([128, H], F32, tag="ps")
                        mm_chain(ps[...], [(ctx[:, k, cc2*128:(cc2+1)*128],
                                            W["inwT"][:, k, 2*H:3*H])
                                           for k in range(KC)])
                        nc.scalar.activation(vtok[:, cc2, :], ps[...], ACT.Copy)
                    PR["vtok"] = vtok

                return PR, [p_mlp1, p_mlp2, p_qk, p_v]

            pmode = int(os.environ.get('KERNEL_PAIRS', '2'))
            if pmode == 2:
                # opposite-half pairs: each head's scores go to its OWN psum
                # tile, so the PE overlaps head B's LDWEIGHTS with head A's
                # matmul (different row groups, different banks - safe)
                PAIRS = [(0, 1), (2, 3), (4, 5), (6, 7)]
            elif pmode == 1:
                PAIRS = [(0, 2), (1, 3), (4, 6), (5, 7)]
            else:
                PAIRS = [(0, 2), (4, 6), (1, 3), (5, 7)]

            def attn_stages(gp, PR):
                """Per-group attention stage emitters for pair gp, already
                interleaved over the pair's two groups."""
                qk, vtok, ctx = PR["qk"], PR["vtok"], PR["ctx"]
                S = {0: {}, 1: {}}

                def s1_scores(g2):
                    cg = slice(g2 * C, (g2 + 1) * C)
                    den_ps = psp.tile([128, 2 * C], F32, tag="ps")
                    PTs = []
                    for p, pair in enumerate(PAIRS):
                        PT = wk.tile([128, CC, 2 * C], DT, tag=f"PT{p}")
                        PTs.append(PT)
                        for kc in range(CC):
                            if expb:
                                # one 2-bank tile: each bank written by one PE
                                # row group (hazard-safe); both heads evicted
                                # by a single strided exp
                                sc2 = scp.tile([128, 2, 512], F32, tag="sc2")
                                for hh, h in enumerate(pair):
                                    off = 64 * (h % 2)
                                    jslot = h // 2
                                    lhsT = qk[off:off+64, KC + jslot,
                                              g2*C + kc*128: g2*C + (kc+1)*128]
                                    rhs = qk[off:off+64, jslot, cg]
                                    nc.tensor.matmul(sc2[:, hh, 0:C], lhsT, rhs,
                                                     start=True, stop=True,
                                                     skip_group_check=True)
                                nc.scalar.activation(PT[:, kc, :],
                                                     sc2[:, :, 0:C], ACT.Exp)
                                continue
                            psh = []
                            for hh, h in enumerate(pair):
                                ps_h = psp.tile([128, C], F32, tag="ps",
                                                name=f"sc{hh}")
                                off = 64 * (h % 2)
                                jslot = h // 2
                                lhsT = qk[off:off+64, KC + jslot,
                                          g2*C + kc*128: g2*C + (kc+1)*128]
                                rhs = qk[off:off+64, jslot, cg]
                                nc.tensor.matmul(ps_h[...], lhsT, rhs,
                                                 start=True, stop=True,
                                                 skip_group_check=True)
                                psh.append(ps_h)
                            for hh in range(2):
                                nc.scalar.activation(PT[:, kc, hh*C:(hh+1)*C],
                                                     psh[hh][...], ACT.Exp)
                        for kc in range(CC):
                            nc.tensor.matmul(den_ps[32*p:32*p+1, :],
                                             ones128[:, 0:1], PT[:, kc, :],
                                             start=(kc == 0), stop=(kc == CC - 1),
                                             skip_group_check=True,
                                             tile_position=(0, 32 * p))
                    S[g2]["PTs"] = PTs
                    S[g2]["den_ps"] = den_ps

                def s2_recip(g2):
                    den_sb = wk.tile([128, 2 * C], F32, tag="den")
                    nc.vector.tensor_copy(den_sb[...], S[g2]["den_ps"][...])
                    inv_sb = wk.tile([128, 2 * C], F32, tag="inv")
                    nc.vector.reciprocal_approx_fast(inv_sb[...], den_sb[...])
                    S[g2]["inv_sb"] = inv_sb

                def s3_sa(g2):
                    inv_sb = S[g2]["inv_sb"]
                    saT = wk.tile([128, KC, C], DT, tag="saT")
                    for p, pair in enumerate(PAIRS):
                        PT = S[g2]["PTs"][p]
                        sa0 = psp.tile([128, C], F32, tag="ps")
                        bc = psp.tile([128, C], F32, tag="ps")
                        for hh, h in enumerate(pair):
                            mm_chain(sa0[64*hh:64*hh+64, :],
                                     [(vtok[:, 2*g2 + kc, 64*h:64*h+64],
                                       PT[:, kc, hh*C:(hh+1)*C])
                                      for kc in range(CC)])
                            nc.tensor.matmul(bc[64*hh:64*hh+64, :],
                                             onesbc[32*p:32*p+1, :],
                                             inv_sb[32*p:32*p+1, hh*C:(hh+1)*C],
                                             start=True, stop=True,
                                             skip_group_check=True,
                                             tile_position=(32 * p, 64 * hh))
                        bc_sb = sm.tile([128, C], F32, tag="bc")
                        nc.scalar.activation(bc_sb[...], bc[...], ACT.Copy)
                        for hh, h in enumerate(pair):
                            o = 64 * (h % 2)
                            nc.vector.tensor_tensor(saT[o:o+64, h // 2, :],
                                                    sa0[64*hh:64*hh+64, :],
                                                    bc_sb[64*hh:64*hh+64, :],
                                                    ALU.mult)
                    S[g2]["saT"] = saT

                def s4_outproj(g2):
                    cg = slice(g2 * C, (g2 + 1) * C)
                    saT = S[g2]["saT"]
                    rT = wk.tile([128, KC, C], DT, tag="rT")
                    r2T = wk.tile([128, KC, C], DT, tag="r2T")
                    for hc in range(KC):
                        ps = psp.tile([128, C], F32, tag="ps")
                        pairs = [(W["outwT"][:, k, hc*128:(hc+1)*128],
                                  saT[:, k, :]) for k in range(KC)]
                        pairs.append((W["outbrow"][0:1, hc*128:(hc+1)*128],
                                      onesrow[0:1, 0:C]))
                        mm_chain(ps[...], pairs)
                        nc.vector.tensor_tensor(rT[:, hc, :], ps[...],
                                                ctx[:, hc, cg], ALU.add)
                        nc.scalar.activation(r2T[:, hc, :], rT[:, hc, :],
                                             ACT.Square)
                    S[g2]["rT"] = rT
                    S[g2]["r2T"] = r2T

                def s5_stats(g2):
                    # both token chunks' LN stats in [128,2]-wide ops; the
                    # sign of t = mu*istd is folded into s6's subtract
                    rT, r2T = S[g2]["rT"], S[g2]["r2T"]
                    stat = psp.tile([128, 4], F32, tag="ps", name="stat")
                    for cc in range(CC):
                        mm_chain(stat[:, cc:cc+1],
                                 [(rT[:, k, cc*128:(cc+1)*128], ones128[:, 0:1])
                                  for k in range(KC)])
                        mm_chain(stat[:, 2+cc:3+cc],
                                 [(r2T[:, k, cc*128:(cc+1)*128], ones128[:, 0:1])
                                  for k in range(KC)])
                    mu = sm.tile([128, 2], F32, tag="mu")
                    nc.vector.tensor_scalar(mu[...], stat[:, 0:2], 1.0 / H, None,
                                            ALU.mult)
                    s1t = sm.tile([128, 2], F32, tag="s1")
                    nc.vector.tensor_scalar(s1t[...], stat[:, 2:4], 1.0 / H, EPS,
                                            ALU.mult, ALU.add)
                    musq = sm.tile([128, 2], F32, tag="musq")
                    nc.scalar.activation(musq[...], mu[...], ACT.Square)
                    vpe = sm.tile([128, 2], F32, tag="vpe")
                    nc.vector.tensor_tensor(vpe[...], s1t[...], musq[...],
                                            ALU.subtract)
                    lnv = sm.tile([128, 2], F32, tag="lnv")
                    nc.scalar.activation(lnv[...], vpe[...], ACT.Ln)
                    istd = sm.tile([128, 2], F32, tag="istd")
                    nc.scalar.activation(istd[...], lnv[...], ACT.Exp, scale=-0.5)
                    t_ = sm.tile([128, 2], F32, tag="t_")
                    nc.vector.tensor_tensor(t_[...], mu[...], istd[...], ALU.mult)
                    S[g2]["istds"] = [istd[:, cc:cc+1] for cc in range(CC)]
                    S[g2]["ts"] = [t_[:, cc:cc+1] for cc in range(CC)]

                def s6_v(g2):
                    rT = S[g2]["rT"]
                    V = wk.tile([128, CC, H], DT, tag="V")
                    for cc in range(CC):
                        ps = psp.tile([128, H], F32, tag="ps")
                        mm_chain(ps[...], [(rT[:, k, cc*128:(cc+1)*128],
                                            W["wvgT"][:, k, :]) for k in range(KC)])
                        tmp1 = sm.tile([128, H], DT, tag="tmp1")
                        nc.scalar.activation(tmp1[...], ps[...], ACT.Copy,
                                             scale=S[g2]["istds"][cc][...])
                        tmp2 = sm.tile([128, H], DT, tag="tmp2")
                        nc.vector.tensor_scalar(tmp2[...], W["kgb"][...],
                                                S[g2]["ts"][cc][...], None,
                                                ALU.mult)
                        if skip_kv2:
                            nc.vector.tensor_tensor(V[:, cc, :], tmp1[...],
                                                    tmp2[...], ALU.subtract)
                        else:
                            tmp3 = sm.tile([128, H], DT, tag="tmp3")
                            nc.vector.tensor_tensor(tmp3[...], tmp1[...],
                                                    tmp2[...], ALU.subtract)
                            nc.vector.tensor_tensor(V[:, cc, :], tmp3[...],
                                                    W["kv2b"][...], ALU.add)
                    S[g2]["V"] = V

                def s7_logits(g2):
                    g = gp * 2 + g2
                    PTc = wk.tile([128, CC, B], DT, tag="PTc")
                    for cc in range(CC):
                        ps = psp.tile([128, B], F32, tag="ps")
                        mm_chain(ps[...],
                                 [(W["xt"][:, kx, g*C + cc*128: g*C + (cc+1)*128],
                                   W2T[:, kx, :]) for kx in range(4)])
                        nc.scalar.activation(PTc[:, cc, :], ps[...], ACT.Exp)
                    S[g2]["PTc"] = PTc

                def s8_out(g2):
                    g = gp * 2 + g2
                    PTc, V = S[g2]["PTc"], S[g2]["V"]
                    for bc2 in range(BC):
                        z0 = psp.tile([128, H], F32, tag="ps")
                        mm_chain(z0[...], [(PTc[:, kc, bc2*128:(bc2+1)*128],
                                            V[:, kc, :]) for kc in range(CC)])
                        bns = sm.tile([128, 6], F32, tag="bns")
                        nc.vector.bn_stats(bns[...], z0[...])
                        ms = sm.tile([128, 2], F32, tag="ms")
                        nc.vector.bn_aggr(ms[...], bns[...])
                        lnv = sm.tile([128, 1], F32, tag="lnvz")
                        nc.scalar.activation(lnv[...], ms[:, 1:2], ACT.Ln,
                                             bias=epsc[...])
                        istd = sm.tile([128, 1], F32, tag="istdz")
                        nc.scalar.activation(istd[...], lnv[...], ACT.Exp,
                                             scale=-0.5)
                        nmi = sm.tile([128, 1], F32, tag="nmi")
                        nc.vector.tensor_scalar(nmi[...], ms[:, 0:1], istd[...],
                                                -1.0, ALU.mult, ALU.mult)
                        if skip_lnog and skip_lnob:
                            o_sb = sm.tile([128, H], F32, tag="osb")
                            nc.scalar.activation(o_sb[...], z0[...], ACT.Identity,
                                                 scale=istd[...], bias=nmi[...])
                        else:
                            t1 = sm.tile([128, H], F32, tag="t1")
                            nc.scalar.activation(t1[...], z0[...], ACT.Identity,
                                                 scale=istd[...], bias=nmi[...])
                            o_sb = sm.tile([128, H], F32, tag="osb")
                            if skip_lnog:
                                nc.vector.tensor_tensor(o_sb[...], t1[...],
                                                        W["lnobb"][...], ALU.add)
                            elif skip_lnob:
                                nc.vector.tensor_tensor(o_sb[...], t1[...],
                                                        W["lnogb"][...], ALU.mult)
                            else:
                                t2 = sm.tile([128, H], F32, tag="t2")
                                nc.vector.tensor_tensor(t2[...], t1[...],
                                                        W["lnogb"][...], ALU.mult)
                                nc.vector.tensor_tensor(o_sb[...], t2[...],
                                                        W["lnobb"][...], ALU.add)
                        nc.sync.dma_start(out=out_ext[bc2*128:(bc2+1)*128, g, :],
                                          in_=o_sb[...])

                out = []
                for stg in (s1_scores, s2_recip, s3_sa, s4_outproj, s5_stats,
                            s6_v, s7_logits, s8_out):
                    out.append(lambda stg=stg: stg(0))
                    out.append(lambda stg=stg: stg(1))
                return out

            def weave(astgs, pstgs):
                """Emit attention chunks with proj stages spread between them."""
                if not pstgs:
                    for a in astgs:
                        a()
                    return
                # insert a proj stage after every ceil(len/|p|) attention chunks
                k = max(1, len(astgs) // (len(pstgs) + 1))
                pi = 0
                for i, a in enumerate(astgs):
                    a()
                    if (i + 1) % k == 0 and pi < len(pstgs):
                        pstgs[pi]()
                        pi += 1
                while pi < len(pstgs):
                    pstgs[pi]()
                    pi += 1

            NPAIR = MLOC // 2
            do_weave = int(os.environ.get('KERNEL_WEAVE', '0'))
            PR_cur, pstg_cur = make_proj(0)
            for fn in pstg_cur:
                fn()
            emit_w2t()
            for gp in range(NPAIR):
                astgs = attn_stages(gp, PR_cur)
                if gp + 1 < NPAIR:
                    PR_cur, pstg_next = make_proj(gp + 1)
                    if do_weave:
                        weave(astgs, pstg_next)
                    else:
                        for a in astgs:
                            a()
                        for fn in pstg_next:
                            fn()
                else:
                    weave(astgs, [])

    nc.finalize()
    return nc



_CACHE = {}


def _get_nc(key):
    if key not in _CACHE:
        _CACHE[key] = _build(*key)
    return _CACHE[key]


def kernel(**inputs):
    shared, xts = _prep(inputs)
    skip_kv2 = bool(np.all(shared["kv2b"] == 0.0))
    skip_lnog = bool(np.all(shared["lnogb"] == 1.0))
    skip_lnob = bool(np.all(shared["lnobb"] == 0.0))
    key = (COMPUTE_DT, skip_kv2, skip_lnog, skip_lnob)
    nc = _get_nc(key)

    np_dt = _np_dt(COMPUTE_DT)
    in_maps = []
    for ci in range(N_CORES):
        m = {}
        for name in _SHAPES:
            arr = xts[ci] if name == "xt" else shared[name]
            pdt = np_dt if name in _DT_PARAMS else np.dtype(np.float32)
            m[name] = np.ascontiguousarray(arr.astype(pdt))
        in_maps.append(m)

    trace = bool(int(os.environ.get("KERNEL_TRACE", "0")))
    # the axon-tunneled device occasionally reports a transient
    # NRT_EXEC_UNIT_UNRECOVERABLE on the first execution after a prior
    # session; a fresh attempt reliably succeeds, so retry a few times
    last_err = None
    for attempt in range(4):
        try:
            res = run_bass_kernel_spmd(nc, in_maps, list(range(N_CORES)),
                                       trace=trace)
            break
        except Exception as e:  # noqa: BLE001
            last_err = e
            import time
            time.sleep(2.0 * (attempt + 1))
    else:
        raise last_err
    kernel.last_results = res
    out = np.concatenate([res.results[ci]["out"] for ci in range(N_CORES)], axis=1)
    return out.astype(np.float32)



# revision 16
# speedup vs baseline: 1.2622x; 1.2622x over previous
"""Trainium2 Bass kernel for nn_ANPToolEncoder (sparse attention encoder).

Sharding: M=64 context groups split across 8 NeuronCores (8 groups each);
the whole network is embarrassingly parallel in M — each core computes
out[:, m_shard, :] and the host concatenates. No collectives.

Layout convention on-chip: activations are kept "feat-major" ([feature
partitions, token free]) so every matmul contraction runs over the
partition axis; softmax denominators for self-attention are computed with
ones-vector matmuls on the PE; the cross-attention softmax denominator is
never computed at all (the final LayerNorm is invariant to per-row scale).
LayerNorm-then-linear (context LN -> V projection) is folded into the V
matmul via host-side weight scaling (wv*g) plus per-token istd/mu
correction terms.
"""

import os
import sys
import numpy as np

for _p in ("/opt/trn_rl_repo", "/root/.axon_site/_ro/trn_rl_repo"):
    if os.path.isdir(_p) and _p not in sys.path:
        sys.path.append(_p)

from concourse import bass, bacc, tile, mybir  # noqa: E402
from concourse.bass_utils import run_bass_kernel_spmd  # noqa: E402

# Pin the ACT function-table chooser to the one set that contains every
# function this kernel uses (exp/ln/relu/square/copy/identity). The default
# greedy chooser ping-pongs between exp_and_others and
# natural_log_exp_and_others, paying a ~1.3us table load dozens of times.
_ACT_PIN = "natural_log_exp_and_others"
_orig_get_act_tables = bacc.get_activation_tables


def _pinned_act_tables(arch):
    t = _orig_get_act_tables(arch)
    return {name: (fns if name == _ACT_PIN else set())
            for name, fns in t.items()}


bacc.get_activation_tables = _pinned_act_tables

B, M, C, DX, H, NH = 256, 64, 256, 512, 512, 8
DH = H // NH
EPS = 1e-5
N_CORES = 8
MLOC = M // N_CORES          # 8 context groups per core
KC = H // 128                # 4 feature chunks of 128
CC = C // 128                # 2 token chunks per group
BC = B // 128                # 2 query chunks

F32 = mybir.dt.float32
BF16 = mybir.dt.bfloat16
ACT = mybir.ActivationFunctionType
ALU = mybir.AluOpType

# compute dtype for matmul-feeding SBUF tensors ("f32" or "bf16")
COMPUTE_DT = os.environ.get("KERNEL_DT", "bf16")


def _np_dt(dt_str):
    if dt_str == "bf16":
        import ml_dtypes
        return np.dtype(ml_dtypes.bfloat16)
    return np.dtype(np.float32)


def _prep(inp):
    """Host-side weight folds + feat-major layouts. Returns (shared_map, per-core key)."""
    f = {k: np.asarray(v, np.float32) for k, v in inp.items()}
    w1 = f["cp_w1"]                                   # [H, DX+2]
    inw, inb = f["in_w"], f["in_b"]
    bq, bk, bv = inb[:H], inb[H:2*H], inb[2*H:]

    def chunkT(w):      # [out, in] -> [128, in/128, out]  (feat-major, k-chunked)
        wT = w.T                                      # [in, out]
        return np.ascontiguousarray(
            wT.reshape(wT.shape[0] // 128, 128, wT.shape[1]).transpose(1, 0, 2))

    def chunkv(v):      # [H] -> [128, KC]
        return np.ascontiguousarray(v.reshape(KC, 128).T)

    sH = 1.0 / np.sqrt(H)
    shared = {
        "w1kT": chunkT(w1[:, :DX]),                   # [128,4,512]
        "w1gT": np.ascontiguousarray(w1[:, DX:DX+2].T),  # [2,512]
        "b1":   chunkv(f["cp_b1"]),
        "w2T":  chunkT(f["cp_w2"]),
        "b2":   chunkv(f["cp_b2"]),
        "inwT": chunkT(inw),                          # [128,4,1536]
        "bq8":  chunkv(bq / np.sqrt(DH)),
        "bk":   chunkv(bk),
        "outwT": chunkT(f["out_w"]),
        "outbrow": (f["out_b"] + f["out_w"] @ bv)[None, :],   # [1,512]
        "wvgT": chunkT(f["wv_w"] * f["lnc_g"][None, :]),
        "kgb":  np.broadcast_to(f["wv_w"] @ f["lnc_g"], (128, H)).copy(),
        "kv2b": np.broadcast_to(f["wv_w"] @ f["lnc_b"] + f["wv_b"], (128, H)).copy(),
        # cross-attn K-proj folded away: logits_T = img @ W2', with
        # W2' = (sH*wk^T wq) @ qe^T + (sH*wk^T wq_b) broadcast; wk_b only
        # scales P~ columns, which the final LN's row-scale invariance eats
        "wkqT": chunkT(sH * f["wk_w"].T @ f["wq_w"]),
        "c2":   chunkv(sH * f["wk_w"].T @ f["wq_b"]),
        "lnogb": np.broadcast_to(f["lno_g"], (128, H)).copy(),
        "lnobb": np.broadcast_to(f["lno_b"], (128, H)).copy(),
        # query embed feat-major: [128, 4, 256]
        "qet": np.ascontiguousarray(
            f["query_embed"].T.reshape(KC, 128, B).transpose(1, 0, 2)),
    }

    # per-core X feat-major, one contiguous param per 128-dim chunk
    # (xt0..xt3 [128, MLOC*C]) plus xtg [2, MLOC*C] rows gt/pred
    img, gt, pr = f["ctx_img_feat"], f["ctx_gt"], f["ctx_pred"]
    xts = []
    for ci in range(N_CORES):
        gs = slice(ci * MLOC, (ci + 1) * MLOC)
        xi = img[gs].reshape(MLOC * C, DX).T          # [512, 2048]
        xk = xi.reshape(4, 128, MLOC * C).transpose(1, 0, 2)
        m = {f"xt{k}": np.ascontiguousarray(xk[:, k, :]) for k in range(4)}
        m["xtg"] = np.ascontiguousarray(
            np.stack([gt[gs].reshape(-1), pr[gs].reshape(-1)]))
        xts.append(m)
    return shared, xts


# names of DT-typed (matmul-feeding) params; everything else stays f32
_DT_PARAMS = {"w1kT", "w1gT", "w2T", "inwT", "outwT", "outbrow", "wvgT",
              "kgb", "kv2b", "wkqT", "lnogb", "lnobb", "qet",
              "xt0", "xt1", "xt2", "xt3", "xtg"}

_SHAPES = {
    "xt0":   [128, MLOC * C], "xt1": [128, MLOC * C],
    "xt2":   [128, MLOC * C], "xt3": [128, MLOC * C],
    "xtg":   [2, MLOC * C],
    "qet":   [128, KC, B],
    "w1kT":  [128, KC, H], "w1gT": [2, H], "b1": [128, KC],
    "w2T":   [128, KC, H], "b2": [128, KC],
    "inwT":  [128, KC, 3 * H], "bq8": [128, KC], "bk": [128, KC],
    "outwT": [128, KC, H], "outbrow": [1, H],
    "wvgT":  [128, KC, H], "kgb": [128, H], "kv2b": [128, H],
    "wkqT":  [128, KC, H], "c2": [128, KC],
    "lnogb": [128, H], "lnobb": [128, H],
}

# output dtype: bf16 halves the 4MB/core writeback (host casts back)
OUT_DT = os.environ.get("KERNEL_OUTDT", "bf16")


def _build(dt_str, skip_kv2, skip_lnog, skip_lnob, stage=99):
    DT = BF16 if dt_str == "bf16" else F32
    nc = bacc.Bacc("TRN2", target_bir_lowering=False, debug=False,
                   num_devices=N_CORES)

    P = {}
    for name, shp in _SHAPES.items():
        pdt = DT if name in _DT_PARAMS else F32
        P[name] = nc.declare_dram_parameter(name, shp, pdt, isOutput=False)
    ODT = BF16 if OUT_DT == "bf16" else F32
    out_ext = nc.declare_dram_parameter("out", [B, MLOC, H], ODT, isOutput=True)

    with tile.TileContext(nc) as tc:
        with tc.tile_pool(name="wt", bufs=1) as wt, \
             tc.tile_pool(name="wk", bufs=2) as wk, \
             tc.tile_pool(name="sm", bufs=3) as sm, \
             tc.tile_pool(name="ps", bufs=4, space="PSUM") as psp, \
             tc.tile_pool(name="sc", bufs=2, space="PSUM") as scp:

            # ---- load weights / consts ----
            unused = set()
            if skip_kv2:
                unused.add("kv2b")
            if skip_lnog:
                unused.add("lnogb")
            if skip_lnob:
                unused.add("lnobb")
            # spread input DMAs over the three DMA-capable queues, ordered
            # by first compute use so mlp1/mlp2/qk of pair 0 and the W2T
            # precompute are never starved; scalar kept light (ACT is busy)
            dma_plan = [
                (nc.scalar, ["w1kT", "b1", "w1gT", "w2T", "b2", "bq8", "bk"]),
                (nc.sync,   ["xt0", "xt1", "xtg", "qet", "wkqT", "c2",
                             "outwT", "outbrow", "wvgT", "kgb",
                             "kv2b", "lnogb", "lnobb"]),
                (nc.gpsimd, ["xt2", "xt3", "inwT"]),
            ]
            W = {}
            for eng, names in dma_plan:
                for name in names:
                    if name in unused:
                        continue
                    pdt = DT if name in _DT_PARAMS else F32
                    t = wt.tile(_SHAPES[name], pdt, tag=name)
                    if name == "inwT":
                        # q+k halves (first use: p_qk) before the v third
                        eng.dma_start(out=t[:, :, 0:2*H], in_=P[name][:, :, 0:2*H])
                        eng.dma_start(out=t[:, :, 2*H:], in_=P[name][:, :, 2*H:])
                    else:
                        eng.dma_start(out=t[...], in_=P[name][...])
                    W[name] = t
            ones128 = wt.tile([128, 1], DT, tag="ones128")
            nc.vector.memset(ones128[:], 1.0)
            onesrow = wt.tile([1, 2 * C], DT, tag="onesrow")
            nc.vector.memset(onesrow[:], 1.0)
            # all-ones stationary: a den matmul with this lhsT yields the
            # key-sum already replicated on every output partition
            ones2d = wt.tile([128, 128], DT, tag="ones2d")
            nc.vector.memset(ones2d[:], 1.0)
            epsc = wt.tile([128, 1], F32, tag="epsc")
            nc.vector.memset(epsc[:], EPS)

            def mm_chain(ps_ap, pairs):
                """Accumulating matmul chain: pairs = [(lhsT, rhs), ...]."""
                n = len(pairs)
                for i, (l, r) in enumerate(pairs):
                    nc.tensor.matmul(ps_ap, l, r, start=(i == 0), stop=(i == n - 1),
                                     skip_group_check=True)

            # ---- W2' = WKQ @ qe^T + c2 (once; replaces Q and K projs).
            # Emitted AFTER pair-0 projections: the in-order PE stream must
            # not start with matmuls that wait on late-arriving DMAs. ----
            W2T = wt.tile([128, KC, B], DT, tag="W2T")

            def emit_w2t():
                for dxc in range(KC):
                    ps = psp.tile([128, B], F32, tag="ps")
                    mm_chain(ps[...], [(W["wkqT"][:, k, dxc*128:(dxc+1)*128],
                                        W["qet"][:, k, :]) for k in range(KC)])
                    nc.scalar.activation(W2T[:, dxc, :], ps[...], ACT.Identity,
                                         bias=W["c2"][:, dxc:dxc+1])

            # ---- software-pipelined group-pair schedule ----
            # Projections for pair gp+1 are woven between the attention
            # stages of pair gp so the PE stream stays dense (HAM warm).

            def make_proj(gp):
                """Emitters for pair gp's batched (N=512) projections."""
                xg2 = slice(gp * 2 * C, (gp + 1) * 2 * C)
                PR = {}

                def p_mlp1():
                    h1 = wk.tile([128, KC, 2 * C], DT, tag="h1")
                    for hc in range(KC):
                        ps = psp.tile([128, 2 * C], F32, tag="ps")
                        pairs = [(W["w1kT"][:, k, hc*128:(hc+1)*128],
                                  W[f"xt{k}"][:, xg2]) for k in range(4)]
                        pairs.append((W["w1gT"][0:2, hc*128:(hc+1)*128],
                                      W["xtg"][0:2, xg2]))
                        mm_chain(ps[...], pairs)
                        nc.scalar.activation(h1[:, hc, :], ps[...], ACT.Relu,
                                             bias=W["b1"][:, hc:hc+1])
                    PR["h1"] = h1

                def p_mlp2():
                    h1 = PR["h1"]
                    ctx = wk.tile([128, KC, 2 * C], DT, tag="ctx")
                    for hc in range(KC):
                        ps = psp.tile([128, 2 * C], F32, tag="ps")
                        mm_chain(ps[...], [(W["w2T"][:, k, hc*128:(hc+1)*128],
                                            h1[:, k, :]) for k in range(KC)])
                        nc.vector.tensor_scalar(ctx[:, hc, :], ps[...],
                                                W["b2"][:, hc:hc+1], None, ALU.add)
                    PR["ctx"] = ctx

                def p_qk():
                    ctx = PR["ctx"]
                    qk = wk.tile([128, 2 * KC, 2 * C], DT, tag="qk")
                    for jc in range(2 * KC):
                        ps = psp.tile([128, 2 * C], F32, tag="ps")
                        mm_chain(ps[...], [(W["inwT"][:, k, jc*128:(jc+1)*128],
                                            ctx[:, k, :]) for k in range(KC)])
                        if jc < KC:
                            nc.vector.tensor_scalar(qk[:, jc, :], ps[...],
                                                    1.0 / float(np.sqrt(DH)),
                                                    W["bq8"][:, jc:jc+1],
                                                    ALU.mult, ALU.add)
                        else:
                            nc.vector.tensor_scalar(qk[:, jc, :], ps[...],
                                                    W["bk"][:, jc-KC:jc-KC+1], None,
                                                    ALU.add)
                    PR["qk"] = qk

                def p_v():
                    ctx = PR["ctx"]
                    vtok = wk.tile([128, 2 * CC, H], DT, tag="vtok")
                    for cc2 in range(2 * CC):
                        ps = psp.tile([128, H], F32, tag="ps")
                        mm_chain(ps[...], [(ctx[:, k, cc2*128:(cc2+1)*128],
                                            W["inwT"][:, k, 2*H:3*H])
                                           for k in range(KC)])
                        nc.scalar.activation(vtok[:, cc2, :], ps[...], ACT.Copy)
                    PR["vtok"] = vtok

                return PR, [p_mlp1, p_mlp2, p_qk, p_v]

            pmode = int(os.environ.get('KERNEL_PAIRS', '2'))
            if pmode == 2:
                # opposite-half pairs: each head's scores go to its OWN psum
                # tile, so the PE overlaps head B's LDWEIGHTS with head A's
                # matmul (different row groups, different banks - safe)
                PAIRS = [(0, 1), (2, 3), (4, 5), (6, 7)]
            elif pmode == 1:
                PAIRS = [(0, 2), (1, 3), (4, 6), (5, 7)]
            else:
                PAIRS = [(0, 2), (4, 6), (1, 3), (5, 7)]

            def attn_stages(gp, PR):
                """Per-group attention stage emitters for pair gp, already
                interleaved over the pair's two groups."""
                qk, vtok, ctx = PR["qk"], PR["vtok"], PR["ctx"]
                S = {0: {}, 1: {}}

                def s1_scores(g2):
                    cg = slice(g2 * C, (g2 + 1) * C)
                    PTs = []
                    for p, pair in enumerate(PAIRS):
                        PT = wk.tile([128, CC, 2 * C], DT, tag=f"PT{p}")
                        PTs.append(PT)
                        for kc in range(CC):
                            # one 2-bank tile: each bank written by one PE
                            # row group (hazard-safe); both heads evicted
                            # by a single strided exp
                            sc2 = scp.tile([128, 2, 512], F32, tag="sc2")
                            for hh, h in enumerate(pair):
                                off = 64 * (h % 2)
                                jslot = h // 2
                                lhsT = qk[off:off+64, KC + jslot,
                                          g2*C + kc*128: g2*C + (kc+1)*128]
                                rhs = qk[off:off+64, jslot, cg]
                                nc.tensor.matmul(sc2[:, hh, 0:C], lhsT, rhs,
                                                 start=True, stop=True,
                                                 skip_group_check=True)
                            nc.scalar.activation(PT[:, kc, :],
                                                 sc2[:, :, 0:C], ACT.Exp)
                    S[g2]["PTs"] = PTs
                    # denominators after all exps are in flight: the all-ones
                    # stationary replicates each key-sum to every partition,
                    # so no separate broadcast step is needed
                    invs = []
                    for p in range(len(PAIRS)):
                        den_ps = psp.tile([128, 2 * C], F32, tag="ps",
                                          name=f"den{p}")
                        for kc in range(CC):
                            nc.tensor.matmul(den_ps[...], ones2d[...],
                                             PTs[p][:, kc, :],
                                             start=(kc == 0), stop=(kc == CC - 1),
                                             skip_group_check=True)
                        inv_bc = wk.tile([128, 2 * C], F32, tag=f"inv{p}")
                        nc.vector.reciprocal_approx_fast(inv_bc[...], den_ps[...])
                        invs.append(inv_bc)
                    S[g2]["invs"] = invs

                def s3_sa(g2):
                    invs = S[g2]["invs"]
                    saT = wk.tile([128, KC, C], DT, tag="saT")
                    for p, pair in enumerate(PAIRS):
                        PT = S[g2]["PTs"][p]
                        sa0 = psp.tile([128, C], F32, tag="ps")
                        for hh, h in enumerate(pair):
                            mm_chain(sa0[64*hh:64*hh+64, :],
                                     [(vtok[:, 2*g2 + kc, 64*h:64*h+64],
                                       PT[:, kc, hh*C:(hh+1)*C])
                                      for kc in range(CC)])
                        for hh, h in enumerate(pair):
                            o = 64 * (h % 2)
                            nc.vector.tensor_tensor(saT[o:o+64, h // 2, :],
                                                    sa0[64*hh:64*hh+64, :],
                                                    invs[p][o:o+64,
                                                            hh*C:(hh+1)*C],
                                                    ALU.mult)
                    S[g2]["saT"] = saT

                def s4_outproj(g2):
                    cg = slice(g2 * C, (g2 + 1) * C)
                    saT = S[g2]["saT"]
                    rT = wk.tile([128, KC, C], DT, tag="rT")
                    r2T = wk.tile([128, KC, C], DT, tag="r2T")
                    for hc in range(KC):
                        ps = psp.tile([128, C], F32, tag="ps")
                        pairs = [(W["outwT"][:, k, hc*128:(hc+1)*128],
                                  saT[:, k, :]) for k in range(KC)]
                        pairs.append((W["outbrow"][0:1, hc*128:(hc+1)*128],
                                      onesrow[0:1, 0:C]))
                        mm_chain(ps[...], pairs)
                        nc.vector.tensor_tensor(rT[:, hc, :], ps[...],
                                                ctx[:, hc, cg], ALU.add)
                        nc.scalar.activation(r2T[:, hc, :], rT[:, hc, :],
                                             ACT.Square)
                    S[g2]["rT"] = rT
                    S[g2]["r2T"] = r2T

                def s5_stats(g2):
                    # both token chunks' LN stats in [128,2]-wide ops; the
                    # sign of t = mu*istd is folded into s6's subtract
                    rT, r2T = S[g2]["rT"], S[g2]["r2T"]
                    stat = psp.tile([128, 4], F32, tag="ps", name="stat")
                    for cc in range(CC):
                        mm_chain(stat[:, cc:cc+1],
                                 [(rT[:, k, cc*128:(cc+1)*128], ones128[:, 0:1])
                                  for k in range(KC)])
                        mm_chain(stat[:, 2+cc:3+cc],
                                 [(r2T[:, k, cc*128:(cc+1)*128], ones128[:, 0:1])
                                  for k in range(KC)])
                    mu = sm.tile([128, 2], F32, tag="mu")
                    nc.vector.tensor_scalar(mu[...], stat[:, 0:2], 1.0 / H, None,
                                            ALU.mult)
                    s1t = sm.tile([128, 2], F32, tag="s1")
                    nc.vector.tensor_scalar(s1t[...], stat[:, 2:4], 1.0 / H, EPS,
                                            ALU.mult, ALU.add)
                    musq = sm.tile([128, 2], F32, tag="musq")
                    nc.scalar.activation(musq[...], mu[...], ACT.Square)
                    vpe = sm.tile([128, 2], F32, tag="vpe")
                    nc.vector.tensor_tensor(vpe[...], s1t[...], musq[...],
                                            ALU.subtract)
                    lnv = sm.tile([128, 2], F32, tag="lnv")
                    nc.scalar.activation(lnv[...], vpe[...], ACT.Ln)
                    istd = sm.tile([128, 2], F32, tag="istd")
                    nc.scalar.activation(istd[...], lnv[...], ACT.Exp, scale=-0.5)
                    t_ = sm.tile([128, 2], F32, tag="t_")
                    nc.vector.tensor_tensor(t_[...], mu[...], istd[...], ALU.mult)
                    S[g2]["istds"] = [istd[:, cc:cc+1] for cc in range(CC)]
                    S[g2]["ts"] = [t_[:, cc:cc+1] for cc in range(CC)]

                def s6_v(g2):
                    rT = S[g2]["rT"]
                    V = wk.tile([128, CC, H], DT, tag="V")
                    for cc in range(CC):
                        ps = psp.tile([128, H], F32, tag="ps")
                        mm_chain(ps[...], [(rT[:, k, cc*128:(cc+1)*128],
                                            W["wvgT"][:, k, :]) for k in range(KC)])
                        tmp1 = sm.tile([128, H], DT, tag="tmp1")
                        nc.scalar.activation(tmp1[...], ps[...], ACT.Copy,
                                             scale=S[g2]["istds"][cc][...])
                        tmp2 = sm.tile([128, H], DT, tag="tmp2")
                        nc.vector.tensor_scalar(tmp2[...], W["kgb"][...],
                                                S[g2]["ts"][cc][...], None,
                                                ALU.mult)
                        if skip_kv2:
                            nc.vector.tensor_tensor(V[:, cc, :], tmp1[...],
                                                    tmp2[...], ALU.subtract)
                        else:
                            tmp3 = sm.tile([128, H], DT, tag="tmp3")
                            nc.vector.tensor_tensor(tmp3[...], tmp1[...],
                                                    tmp2[...], ALU.subtract)
                            nc.vector.tensor_tensor(V[:, cc, :], tmp3[...],
                                                    W["kv2b"][...], ALU.add)
                    S[g2]["V"] = V

                def s7_logits(g2):
                    g = gp * 2 + g2
                    PTc = wk.tile([128, CC, B], DT, tag="PTc")
                    for cc in range(CC):
                        ps = psp.tile([128, B], F32, tag="ps")
                        mm_chain(ps[...],
                                 [(W[f"xt{kx}"][:, g*C + cc*128: g*C + (cc+1)*128],
                                   W2T[:, kx, :]) for kx in range(4)])
                        nc.scalar.activation(PTc[:, cc, :], ps[...], ACT.Exp)
                    S[g2]["PTc"] = PTc

                def s8_out(g2):
                    g = gp * 2 + g2
                    out_engs = [nc.sync, nc.gpsimd, nc.scalar]
                    PTc, V = S[g2]["PTc"], S[g2]["V"]
                    for bc2 in range(BC):
                        z0 = psp.tile([128, H], F32, tag="ps")
                        mm_chain(z0[...], [(PTc[:, kc, bc2*128:(bc2+1)*128],
                                            V[:, kc, :]) for kc in range(CC)])
                        bns = sm.tile([128, 6], F32, tag="bns")
                        nc.vector.bn_stats(bns[...], z0[...])
                        ms = sm.tile([128, 2], F32, tag="ms")
                        nc.vector.bn_aggr(ms[...], bns[...])
                        lnv = sm.tile([128, 1], F32, tag="lnvz")
                        nc.scalar.activation(lnv[...], ms[:, 1:2], ACT.Ln,
                                             bias=epsc[...])
                        istd = sm.tile([128, 1], F32, tag="istdz")
                        nc.scalar.activation(istd[...], lnv[...], ACT.Exp,
                                             scale=-0.5)
                        nmi = sm.tile([128, 1], F32, tag="nmi")
                        nc.vector.tensor_scalar(nmi[...], ms[:, 0:1], istd[...],
                                                -1.0, ALU.mult, ALU.mult)
                        if skip_lnog and skip_lnob:
                            o_sb = sm.tile([128, H], ODT, tag="osb")
                            nc.scalar.activation(o_sb[...], z0[...], ACT.Identity,
                                                 scale=istd[...], bias=nmi[...])
                        else:
                            t1 = sm.tile([128, H], F32, tag="t1")
                            nc.scalar.activation(t1[...], z0[...], ACT.Identity,
                                                 scale=istd[...], bias=nmi[...])
                            o_sb = sm.tile([128, H], ODT, tag="osb")
                            if skip_lnog:
                                nc.vector.tensor_tensor(o_sb[...], t1[...],
                                                        W["lnobb"][...], ALU.add)
                            elif skip_lnob:
                                nc.vector.tensor_tensor(o_sb[...], t1[...],
                                                        W["lnogb"][...], ALU.mult)
                            else:
                                t2 = sm.tile([128, H], F32, tag="t2")
                                nc.vector.tensor_tensor(t2[...], t1[...],
                                                        W["lnogb"][...], ALU.mult)
                                nc.vector.tensor_tensor(o_sb[...], t2[...],
                                                        W["lnobb"][...], ALU.add)
                        eng = out_engs[(g * BC + bc2) % len(out_engs)]
                        eng.dma_start(out=out_ext[bc2*128:(bc2+1)*128, g, :],
                                      in_=o_sb[...])

                out = []
                for stg in (s1_scores, s3_sa, s4_outproj, s5_stats,
                            s6_v, s7_logits, s8_out):
                    out.append(lambda stg=stg: stg(0))
                    out.append(lambda stg=stg: stg(1))
                return out

            def weave(astgs, pstgs):
                """Emit attention chunks with proj stages spread between them."""
                if not pstgs:
                    for a in astgs:
                        a()
                    return
                # insert a proj stage after every ceil(len/|p|) attention chunks
                k = max(1, len(astgs) // (len(pstgs) + 1))
                pi = 0
                for i, a in enumerate(astgs):
                    a()
                    if (i + 1) % k == 0 and pi < len(pstgs):
                        pstgs[pi]()
                        pi += 1
                while pi < len(pstgs):
                    pstgs[pi]()
                    pi += 1

            NPAIR = MLOC // 2
            do_weave = int(os.environ.get('KERNEL_WEAVE', '0'))
            PR_cur, pstg_cur = make_proj(0)
            for fn in pstg_cur:
                fn()
            emit_w2t()
            for gp in range(NPAIR):
                astgs = attn_stages(gp, PR_cur)
                if gp + 1 < NPAIR:
                    PR_cur, pstg_next = make_proj(gp + 1)
                    if do_weave:
                        weave(astgs, pstg_next)
                    else:
                        for a in astgs:
                            a()
                        for fn in pstg_next:
                            fn()
                else:
                    weave(astgs, [])

    nc.finalize()
    return nc



_CACHE = {}


def _get_nc(key):
    if key not in _CACHE:
        _CACHE[key] = _build(*key)
    return _CACHE[key]


def kernel(**inputs):
    shared, xts = _prep(inputs)
    skip_kv2 = bool(np.all(shared["kv2b"] == 0.0))
    skip_lnog = bool(np.all(shared["lnogb"] == 1.0))
    skip_lnob = bool(np.all(shared["lnobb"] == 0.0))
    key = (COMPUTE_DT, skip_kv2, skip_lnog, skip_lnob)
    nc = _get_nc(key)

    np_dt = _np_dt(COMPUTE_DT)
    in_maps = []
    for ci in range(N_CORES):
        m = {}
        for name in _SHAPES:
            arr = xts[ci][name] if name in xts[ci] else shared[name]
            pdt = np_dt if name in _DT_PARAMS else np.dtype(np.float32)
            m[name] = np.ascontiguousarray(arr.astype(pdt))
        in_maps.append(m)

    trace = bool(int(os.environ.get("KERNEL_TRACE", "0")))
    # the axon-tunneled device occasionally reports a transient
    # NRT_EXEC_UNIT_UNRECOVERABLE on the first execution after a prior
    # session; a fresh attempt reliably succeeds, so retry a few times
    last_err = None
    for attempt in range(4):
        try:
            res = run_bass_kernel_spmd(nc, in_maps, list(range(N_CORES)),
                                       trace=trace)
            break
        except Exception as e:  # noqa: BLE001
            last_err = e
            import time
            time.sleep(2.0 * (attempt + 1))
    else:
        raise last_err
    kernel.last_results = res
    out = np.concatenate([res.results[ci]["out"] for ci in range(N_CORES)], axis=1)
    return out.astype(np.float32)



# revision 19
# speedup vs baseline: 1.6093x; 1.2750x over previous
"""Trainium2 Bass kernel for nn_ANPToolEncoder (sparse attention encoder).

Sharding: M=64 context groups split across 8 NeuronCores (8 groups each);
the whole network is embarrassingly parallel in M — each core computes
out[:, m_shard, :] and the host concatenates. No collectives.

Layout convention on-chip: activations are kept "feat-major" ([feature
partitions, token free]) so every matmul contraction runs over the
partition axis; softmax denominators for self-attention are computed with
ones-vector matmuls on the PE; the cross-attention softmax denominator is
never computed at all (the final LayerNorm is invariant to per-row scale).
LayerNorm-then-linear (context LN -> V projection) is folded into the V
matmul via host-side weight scaling (wv*g) plus per-token istd/mu
correction terms.
"""

import os
import sys
import numpy as np

for _p in ("/opt/trn_rl_repo", "/root/.axon_site/_ro/trn_rl_repo"):
    if os.path.isdir(_p) and _p not in sys.path:
        sys.path.append(_p)

from concourse import bass, bacc, tile, mybir  # noqa: E402
from concourse.bass_utils import run_bass_kernel_spmd  # noqa: E402

# Pin the ACT function-table chooser to the one set that contains every
# function this kernel uses (exp/ln/relu/square/copy/identity). The default
# greedy chooser ping-pongs between exp_and_others and
# natural_log_exp_and_others, paying a ~1.3us table load dozens of times.
_ACT_PIN = "natural_log_exp_and_others"
_orig_get_act_tables = bacc.get_activation_tables


def _pinned_act_tables(arch):
    t = _orig_get_act_tables(arch)
    return {name: (fns if name == _ACT_PIN else set())
            for name, fns in t.items()}


bacc.get_activation_tables = _pinned_act_tables

B, M, C, DX, H, NH = 256, 64, 256, 512, 512, 8
DH = H // NH
EPS = 1e-5
N_CORES = 8
MLOC = M // N_CORES          # 8 context groups per core
KC = H // 128                # 4 feature chunks of 128
CC = C // 128                # 2 token chunks per group
BC = B // 128                # 2 query chunks

F32 = mybir.dt.float32
BF16 = mybir.dt.bfloat16
ACT = mybir.ActivationFunctionType
ALU = mybir.AluOpType

# compute dtype for matmul-feeding SBUF tensors ("f32" or "bf16")
COMPUTE_DT = os.environ.get("KERNEL_DT", "bf16")


def _np_dt(dt_str):
    if dt_str == "bf16":
        import ml_dtypes
        return np.dtype(ml_dtypes.bfloat16)
    return np.dtype(np.float32)


def _prep(inp):
    """Host-side weight folds + feat-major layouts. Returns (shared_map, per-core key)."""
    f = {k: np.asarray(v, np.float32) for k, v in inp.items()}
    w1 = f["cp_w1"]                                   # [H, DX+2]
    inw, inb = f["in_w"], f["in_b"]
    bq, bk, bv = inb[:H], inb[H:2*H], inb[2*H:]

    def chunkT(w):      # [out, in] -> [128, in/128, out]  (feat-major, k-chunked)
        wT = w.T                                      # [in, out]
        return np.ascontiguousarray(
            wT.reshape(wT.shape[0] // 128, 128, wT.shape[1]).transpose(1, 0, 2))

    def chunkv(v):      # [H] -> [128, KC]
        return np.ascontiguousarray(v.reshape(KC, 128).T)

    sH = 1.0 / np.sqrt(H)
    shared = {
        "w1kT": chunkT(w1[:, :DX]),                   # [128,4,512]
        "w1gT": np.ascontiguousarray(w1[:, DX:DX+2].T),  # [2,512]
        "b1":   chunkv(f["cp_b1"]),
        "w2T":  chunkT(f["cp_w2"]),
        "b2":   chunkv(f["cp_b2"]),
        "inwT": chunkT(inw),                          # [128,4,1536]
        "bq8":  chunkv(bq / np.sqrt(DH)),
        "bk":   chunkv(bk),
        "outwT": chunkT(f["out_w"]),
        "outbcol": chunkv(f["out_b"] + f["out_w"] @ bv),      # [128,KC]
        "wvgT": chunkT(f["wv_w"] * f["lnc_g"][None, :]),
        "kgb":  np.broadcast_to(f["wv_w"] @ f["lnc_g"], (128, H)).copy(),
        "kv2b": np.broadcast_to(f["wv_w"] @ f["lnc_b"] + f["wv_b"], (128, H)).copy(),
        # cross-attn K-proj folded away: logits_T = img @ W2', with
        # W2' = (sH*wk^T wq) @ qe^T + (sH*wk^T wq_b) broadcast; wk_b only
        # scales P~ columns, which the final LN's row-scale invariance eats
        "wkqT": chunkT(sH * f["wk_w"].T @ f["wq_w"]),
        "c2":   chunkv(sH * f["wk_w"].T @ f["wq_b"]),
        "lnogb": np.broadcast_to(f["lno_g"], (128, H)).copy(),
        "lnobb": np.broadcast_to(f["lno_b"], (128, H)).copy(),
        # query embed feat-major: [128, 4, 256]
        "qet": np.ascontiguousarray(
            f["query_embed"].T.reshape(KC, 128, B).transpose(1, 0, 2)),
    }

    # per-core X feat-major, one contiguous param per 128-dim chunk
    # (xt0..xt3 [128, MLOC*C]) plus xtg [2, MLOC*C] rows gt/pred
    img, gt, pr = f["ctx_img_feat"], f["ctx_gt"], f["ctx_pred"]
    xts = []
    for ci in range(N_CORES):
        gs = slice(ci * MLOC, (ci + 1) * MLOC)
        xi = img[gs].reshape(MLOC * C, DX).T          # [512, 2048]
        xk = xi.reshape(4, 128, MLOC * C).transpose(1, 0, 2)
        m = {f"xt{k}": np.ascontiguousarray(xk[:, k, :]) for k in range(4)}
        m["xtg"] = np.ascontiguousarray(
            np.stack([gt[gs].reshape(-1), pr[gs].reshape(-1)]))
        xts.append(m)
    return shared, xts


# names of DT-typed (matmul-feeding) params; everything else stays f32
_DT_PARAMS = {"w1kT", "w1gT", "w2T", "inwT", "outwT", "wvgT",
              "kgb", "kv2b", "wkqT", "lnogb", "lnobb", "qet",
              "xt0", "xt1", "xt2", "xt3", "xtg"}

_SHAPES = {
    "xt0":   [128, MLOC * C], "xt1": [128, MLOC * C],
    "xt2":   [128, MLOC * C], "xt3": [128, MLOC * C],
    "xtg":   [2, MLOC * C],
    "qet":   [128, KC, B],
    "w1kT":  [128, KC, H], "w1gT": [2, H], "b1": [128, KC],
    "w2T":   [128, KC, H], "b2": [128, KC],
    "inwT":  [128, KC, 3 * H], "bq8": [128, KC], "bk": [128, KC],
    "outwT": [128, KC, H], "outbcol": [128, KC],
    "wvgT":  [128, KC, H], "kgb": [128, H], "kv2b": [128, H],
    "wkqT":  [128, KC, H], "c2": [128, KC],
    "lnogb": [128, H], "lnobb": [128, H],
}

# output dtype: bf16 halves the 4MB/core writeback (host casts back)
OUT_DT = os.environ.get("KERNEL_OUTDT", "bf16")


def _build(dt_str, skip_kv2, skip_lnog, skip_lnob, stage=99):
    DT = BF16 if dt_str == "bf16" else F32
    nc = bacc.Bacc("TRN2", target_bir_lowering=False, debug=False,
                   num_devices=N_CORES)

    P = {}
    for name, shp in _SHAPES.items():
        pdt = DT if name in _DT_PARAMS else F32
        P[name] = nc.declare_dram_parameter(name, shp, pdt, isOutput=False)
    ODT = BF16 if OUT_DT == "bf16" else F32
    out_ext = nc.declare_dram_parameter("out", [B, MLOC, H], ODT, isOutput=True)

    with tile.TileContext(nc) as tc:
        with tc.tile_pool(name="wt", bufs=1) as wt, \
             tc.tile_pool(name="wk", bufs=2) as wk, \
             tc.tile_pool(name="sm", bufs=3) as sm, \
             tc.tile_pool(name="ps", bufs=4, space="PSUM") as psp, \
             tc.tile_pool(name="sc", bufs=2, space="PSUM") as scp:

            # ---- load weights / consts ----
            unused = set()
            if skip_kv2:
                unused.add("kv2b")
            if skip_lnog:
                unused.add("lnogb")
            if skip_lnob:
                unused.add("lnobb")
            # spread input DMAs over the three DMA-capable queues, ordered
            # by first compute use so mlp1/mlp2/qk of pair 0 and the W2T
            # precompute are never starved; scalar kept light (ACT is busy)
            dma_plan = [
                (nc.scalar, ["w1kT", "b1", "w1gT", "w2T", "b2", "bq8", "bk"]),
                (nc.sync,   ["xt0", "xt1", "xtg", "qet", "wkqT", "c2",
                             "outwT", "outbcol", "wvgT", "kgb",
                             "kv2b", "lnogb", "lnobb"]),
                (nc.gpsimd, ["xt2", "xt3", "inwT"]),
            ]
            W = {}
            for eng, names in dma_plan:
                for name in names:
                    if name in unused:
                        continue
                    pdt = DT if name in _DT_PARAMS else F32
                    t = wt.tile(_SHAPES[name], pdt, tag=name)
                    if name == "inwT":
                        # q+k halves (first use: p_qk) before the v third
                        eng.dma_start(out=t[:, :, 0:2*H], in_=P[name][:, :, 0:2*H])
                        eng.dma_start(out=t[:, :, 2*H:], in_=P[name][:, :, 2*H:])
                    else:
                        eng.dma_start(out=t[...], in_=P[name][...])
                    W[name] = t
            ones128 = wt.tile([128, 1], DT, tag="ones128")
            nc.vector.memset(ones128[:], 1.0)
            # all-ones stationary: a den matmul with this lhsT yields the
            # key-sum already replicated on every output partition
            ones2d = wt.tile([128, 128], DT, tag="ones2d")
            nc.vector.memset(ones2d[:], 1.0)
            epsc = wt.tile([128, 1], F32, tag="epsc")
            nc.vector.memset(epsc[:], EPS)

            def mm_chain(ps_ap, pairs):
                """Accumulating matmul chain: pairs = [(lhsT, rhs), ...]."""
                n = len(pairs)
                for i, (l, r) in enumerate(pairs):
                    nc.tensor.matmul(ps_ap, l, r, start=(i == 0), stop=(i == n - 1),
                                     skip_group_check=True)

            # ---- W2' = WKQ @ qe^T + c2 (once; replaces Q and K projs).
            # Emitted AFTER pair-0 projections: the in-order PE stream must
            # not start with matmuls that wait on late-arriving DMAs. ----
            W2T = wt.tile([128, KC, B], DT, tag="W2T")

            def emit_w2t():
                for dxc in range(KC):
                    ps = psp.tile([128, B], F32, tag="ps")
                    mm_chain(ps[...], [(W["wkqT"][:, k, dxc*128:(dxc+1)*128],
                                        W["qet"][:, k, :]) for k in range(KC)])
                    nc.scalar.activation(W2T[:, dxc, :], ps[...], ACT.Identity,
                                         bias=W["c2"][:, dxc:dxc+1])

            # ---- software-pipelined group-pair schedule ----
            # Projections for pair gp+1 are woven between the attention
            # stages of pair gp so the PE stream stays dense (HAM warm).

            def make_proj(gp):
                """Emitters for pair gp's batched (N=512) projections."""
                xg2 = slice(gp * 2 * C, (gp + 1) * 2 * C)
                PR = {}

                def p_mlp1():
                    h1 = wk.tile([128, KC, 2 * C], DT, tag="h1")
                    for hc in range(KC):
                        ps = psp.tile([128, 2 * C], F32, tag="ps")
                        pairs = [(W["w1kT"][:, k, hc*128:(hc+1)*128],
                                  W[f"xt{k}"][:, xg2]) for k in range(4)]
                        pairs.append((W["w1gT"][0:2, hc*128:(hc+1)*128],
                                      W["xtg"][0:2, xg2]))
                        mm_chain(ps[...], pairs)
                        nc.scalar.activation(h1[:, hc, :], ps[...], ACT.Relu,
                                             bias=W["b1"][:, hc:hc+1])
                    PR["h1"] = h1

                def p_mlp2():
                    h1 = PR["h1"]
                    ctx = wk.tile([128, KC, 2 * C], DT, tag="ctx")
                    for hc in range(KC):
                        ps = psp.tile([128, 2 * C], F32, tag="ps")
                        mm_chain(ps[...], [(W["w2T"][:, k, hc*128:(hc+1)*128],
                                            h1[:, k, :]) for k in range(KC)])
                        nc.vector.tensor_scalar(ctx[:, hc, :], ps[...],
                                                W["b2"][:, hc:hc+1], None, ALU.add)
                    PR["ctx"] = ctx

                def p_qk():
                    ctx = PR["ctx"]
                    qk = wk.tile([128, 2 * KC, 2 * C], DT, tag="qk")
                    for jc in range(2 * KC):
                        ps = psp.tile([128, 2 * C], F32, tag="ps")
                        mm_chain(ps[...], [(W["inwT"][:, k, jc*128:(jc+1)*128],
                                            ctx[:, k, :]) for k in range(KC)])
                        if jc < KC:
                            nc.vector.tensor_scalar(qk[:, jc, :], ps[...],
                                                    1.0 / float(np.sqrt(DH)),
                                                    W["bq8"][:, jc:jc+1],
                                                    ALU.mult, ALU.add)
                        else:
                            nc.vector.tensor_scalar(qk[:, jc, :], ps[...],
                                                    W["bk"][:, jc-KC:jc-KC+1], None,
                                                    ALU.add)
                    PR["qk"] = qk

                def p_v():
                    ctx = PR["ctx"]
                    vtok = wk.tile([128, 2 * CC, H], DT, tag="vtok")
                    for cc2 in range(2 * CC):
                        ps = psp.tile([128, H], F32, tag="ps")
                        mm_chain(ps[...], [(ctx[:, k, cc2*128:(cc2+1)*128],
                                            W["inwT"][:, k, 2*H:3*H])
                                           for k in range(KC)])
                        nc.scalar.activation(vtok[:, cc2, :], ps[...], ACT.Copy)
                    PR["vtok"] = vtok

                return PR, [p_mlp1, p_mlp2, p_qk, p_v]

            pmode = int(os.environ.get('KERNEL_PAIRS', '2'))
            if pmode == 2:
                # opposite-half pairs: each head's scores go to its OWN psum
                # tile, so the PE overlaps head B's LDWEIGHTS with head A's
                # matmul (different row groups, different banks - safe)
                PAIRS = [(0, 1), (2, 3), (4, 5), (6, 7)]
            elif pmode == 1:
                PAIRS = [(0, 2), (1, 3), (4, 6), (5, 7)]
            else:
                PAIRS = [(0, 2), (4, 6), (1, 3), (5, 7)]

            def attn_stages(gp, PR):
                """Per-group attention stage emitters for pair gp, already
                interleaved over the pair's two groups."""
                qk, vtok, ctx = PR["qk"], PR["vtok"], PR["ctx"]
                S = {0: {}, 1: {}}

                def s1_scores(g2):
                    cg = slice(g2 * C, (g2 + 1) * C)
                    PTs = []
                    for p, pair in enumerate(PAIRS):
                        PT = wk.tile([128, CC, 2 * C], DT, tag=f"PT{p}")
                        PTs.append(PT)
                        for kc in range(CC):
                            # one 2-bank tile: each bank written by one PE
                            # row group (hazard-safe); both heads evicted
                            # by a single strided exp
                            sc2 = scp.tile([128, 2, 512], F32, tag="sc2")
                            for hh, h in enumerate(pair):
                                off = 64 * (h % 2)
                                jslot = h // 2
                                lhsT = qk[off:off+64, KC + jslot,
                                          g2*C + kc*128: g2*C + (kc+1)*128]
                                rhs = qk[off:off+64, jslot, cg]
                                nc.tensor.matmul(sc2[:, hh, 0:C], lhsT, rhs,
                                                 start=True, stop=True,
                                                 skip_group_check=True)
                            nc.scalar.activation(PT[:, kc, :],
                                                 sc2[:, :, 0:C], ACT.Exp)
                    S[g2]["PTs"] = PTs
                    # denominators after all exps are in flight: the all-ones
                    # stationary replicates each key-sum to every partition,
                    # so no separate broadcast step is needed
                    invs = []
                    for p in range(len(PAIRS)):
                        den_ps = psp.tile([128, 2 * C], F32, tag="ps",
                                          name=f"den{p}")
                        for kc in range(CC):
                            nc.tensor.matmul(den_ps[...], ones2d[...],
                                             PTs[p][:, kc, :],
                                             start=(kc == 0), stop=(kc == CC - 1),
                                             skip_group_check=True)
                        inv_bc = wk.tile([128, 2 * C], F32, tag=f"inv{p}")
                        nc.vector.reciprocal_approx_fast(inv_bc[...], den_ps[...])
                        invs.append(inv_bc)
                    S[g2]["invs"] = invs

                def s3_sa(g2):
                    invs = S[g2]["invs"]
                    saT = wk.tile([128, KC, C], DT, tag="saT")
                    for p, pair in enumerate(PAIRS):
                        PT = S[g2]["PTs"][p]
                        sa0 = psp.tile([128, C], F32, tag="ps")
                        for hh, h in enumerate(pair):
                            mm_chain(sa0[64*hh:64*hh+64, :],
                                     [(vtok[:, 2*g2 + kc, 64*h:64*h+64],
                                       PT[:, kc, hh*C:(hh+1)*C])
                                      for kc in range(CC)])
                        for hh, h in enumerate(pair):
                            o = 64 * (h % 2)
                            nc.vector.tensor_tensor(saT[o:o+64, h // 2, :],
                                                    sa0[64*hh:64*hh+64, :],
                                                    invs[p][o:o+64,
                                                            hh*C:(hh+1)*C],
                                                    ALU.mult)
                    S[g2]["saT"] = saT

                def s4_outproj(g2):
                    cg = slice(g2 * C, (g2 + 1) * C)
                    saT = S[g2]["saT"]
                    rT = wk.tile([128, KC, C], DT, tag="rT")
                    r2T = wk.tile([128, KC, C], DT, tag="r2T")
                    for hc in range(KC):
                        ps = psp.tile([128, C], F32, tag="ps")
                        mm_chain(ps[...], [(W["outwT"][:, k, hc*128:(hc+1)*128],
                                            saT[:, k, :]) for k in range(KC)])
                        # bias folded in as a per-partition scalar: saves the
                        # K=1 bias-row matmul on the PE
                        nc.vector.scalar_tensor_tensor(
                            rT[:, hc, :], ps[...], W["outbcol"][:, hc:hc+1],
                            ctx[:, hc, cg], ALU.add, ALU.add)
                        nc.scalar.activation(r2T[:, hc, :], rT[:, hc, :],
                                             ACT.Square)
                    S[g2]["rT"] = rT
                    S[g2]["r2T"] = r2T

                def s5_stats(g2):
                    # both token chunks' LN stats in [128,2]-wide ops; the
                    # sign of t = mu*istd is folded into s6's subtract
                    rT, r2T = S[g2]["rT"], S[g2]["r2T"]
                    stat = psp.tile([128, 4], F32, tag="ps", name="stat")
                    for cc in range(CC):
                        mm_chain(stat[:, cc:cc+1],
                                 [(rT[:, k, cc*128:(cc+1)*128], ones128[:, 0:1])
                                  for k in range(KC)])
                        mm_chain(stat[:, 2+cc:3+cc],
                                 [(r2T[:, k, cc*128:(cc+1)*128], ones128[:, 0:1])
                                  for k in range(KC)])
                    mu = sm.tile([128, 2], F32, tag="mu")
                    nc.vector.tensor_scalar(mu[...], stat[:, 0:2], 1.0 / H, None,
                                            ALU.mult)
                    s1t = sm.tile([128, 2], F32, tag="s1")
                    nc.vector.tensor_scalar(s1t[...], stat[:, 2:4], 1.0 / H, EPS,
                                            ALU.mult, ALU.add)
                    musq = sm.tile([128, 2], F32, tag="musq")
                    nc.scalar.activation(musq[...], mu[...], ACT.Square)
                    vpe = sm.tile([128, 2], F32, tag="vpe")
                    nc.vector.tensor_tensor(vpe[...], s1t[...], musq[...],
                                            ALU.subtract)
                    lnv = sm.tile([128, 2], F32, tag="lnv")
                    nc.scalar.activation(lnv[...], vpe[...], ACT.Ln)
                    istd = sm.tile([128, 2], F32, tag="istd")
                    nc.scalar.activation(istd[...], lnv[...], ACT.Exp, scale=-0.5)
                    t_ = sm.tile([128, 2], F32, tag="t_")
                    nc.vector.tensor_tensor(t_[...], mu[...], istd[...], ALU.mult)
                    S[g2]["istds"] = [istd[:, cc:cc+1] for cc in range(CC)]
                    S[g2]["ts"] = [t_[:, cc:cc+1] for cc in range(CC)]

                def s6_v(g2):
                    rT = S[g2]["rT"]
                    V = wk.tile([128, CC, H], DT, tag="V")
                    for cc in range(CC):
                        ps = psp.tile([128, H], F32, tag="ps")
                        mm_chain(ps[...], [(rT[:, k, cc*128:(cc+1)*128],
                                            W["wvgT"][:, k, :]) for k in range(KC)])
                        tmp1 = sm.tile([128, H], DT, tag="tmp1")
                        nc.scalar.activation(tmp1[...], ps[...], ACT.Copy,
                                             scale=S[g2]["istds"][cc][...])
                        tmp2 = sm.tile([128, H], DT, tag="tmp2")
                        nc.vector.tensor_scalar(tmp2[...], W["kgb"][...],
                                                S[g2]["ts"][cc][...], None,
                                                ALU.mult)
                        if skip_kv2:
                            nc.vector.tensor_tensor(V[:, cc, :], tmp1[...],
                                                    tmp2[...], ALU.subtract)
                        else:
                            tmp3 = sm.tile([128, H], DT, tag="tmp3")
                            nc.vector.tensor_tensor(tmp3[...], tmp1[...],
                                                    tmp2[...], ALU.subtract)
                            nc.vector.tensor_tensor(V[:, cc, :], tmp3[...],
                                                    W["kv2b"][...], ALU.add)
                    S[g2]["V"] = V

                def s7_logits(g2):
                    g = gp * 2 + g2
                    PTc = wk.tile([128, CC, B], DT, tag="PTc")
                    for cc in range(CC):
                        ps = psp.tile([128, B], F32, tag="ps")
                        mm_chain(ps[...],
                                 [(W[f"xt{kx}"][:, g*C + cc*128: g*C + (cc+1)*128],
                                   W2T[:, kx, :]) for kx in range(4)])
                        nc.scalar.activation(PTc[:, cc, :], ps[...], ACT.Exp)
                    S[g2]["PTc"] = PTc

                def s8_out(g2):
                    g = gp * 2 + g2
                    out_engs = [nc.sync, nc.gpsimd, nc.scalar]
                    PTc, V = S[g2]["PTc"], S[g2]["V"]
                    for bc2 in range(BC):
                        z0 = psp.tile([128, H], F32, tag="ps")
                        mm_chain(z0[...], [(PTc[:, kc, bc2*128:(bc2+1)*128],
                                            V[:, kc, :]) for kc in range(CC)])
                        bns = sm.tile([128, 6], F32, tag="bns")
                        nc.vector.bn_stats(bns[...], z0[...])
                        ms = sm.tile([128, 2], F32, tag="ms")
                        nc.vector.bn_aggr(ms[...], bns[...])
                        lnv = sm.tile([128, 1], F32, tag="lnvz")
                        nc.scalar.activation(lnv[...], ms[:, 1:2], ACT.Ln,
                                             bias=epsc[...])
                        istd = sm.tile([128, 1], F32, tag="istdz")
                        nc.scalar.activation(istd[...], lnv[...], ACT.Exp,
                                             scale=-0.5)
                        nmi = sm.tile([128, 1], F32, tag="nmi")
                        nc.vector.tensor_scalar(nmi[...], ms[:, 0:1], istd[...],
                                                -1.0, ALU.mult, ALU.mult)
                        if skip_lnog and skip_lnob:
                            o_sb = sm.tile([128, H], ODT, tag="osb")
                            nc.scalar.activation(o_sb[...], z0[...], ACT.Identity,
                                                 scale=istd[...], bias=nmi[...])
                        else:
                            t1 = sm.tile([128, H], F32, tag="t1")
                            nc.scalar.activation(t1[...], z0[...], ACT.Identity,
                                                 scale=istd[...], bias=nmi[...])
                            o_sb = sm.tile([128, H], ODT, tag="osb")
                            if skip_lnog:
                                nc.vector.tensor_tensor(o_sb[...], t1[...],
                                                        W["lnobb"][...], ALU.add)
                            elif skip_lnob:
                                nc.vector.tensor_tensor(o_sb[...], t1[...],
                                                        W["lnogb"][...], ALU.mult)
                            else:
                                t2 = sm.tile([128, H], F32, tag="t2")
                                nc.vector.tensor_tensor(t2[...], t1[...],
                                                        W["lnogb"][...], ALU.mult)
                                nc.vector.tensor_tensor(o_sb[...], t2[...],
                                                        W["lnobb"][...], ALU.add)
                        eng = out_engs[(g * BC + bc2) % len(out_engs)]
                        eng.dma_start(out=out_ext[bc2*128:(bc2+1)*128, g, :],
                                      in_=o_sb[...])

                out = []
                for stg in (s1_scores, s3_sa, s4_outproj, s5_stats,
                            s6_v, s7_logits, s8_out):
                    out.append(lambda stg=stg: stg(0))
                    out.append(lambda stg=stg: stg(1))
                return out

            def weave(astgs, pstgs):
                """Emit attention chunks with proj stages spread between them."""
                if not pstgs:
                    for a in astgs:
                        a()
                    return
                # insert a proj stage after every ceil(len/|p|) attention chunks
                k = max(1, len(astgs) // (len(pstgs) + 1))
                pi = 0
                for i, a in enumerate(astgs):
                    a()
                    if (i + 1) % k == 0 and pi < len(pstgs):
                        pstgs[pi]()
                        pi += 1
                while pi < len(pstgs):
                    pstgs[pi]()
                    pi += 1

            NPAIR = MLOC // 2
            do_weave = int(os.environ.get('KERNEL_WEAVE', '0'))
            PR_cur, pstg_cur = make_proj(0)
            for fn in pstg_cur:
                fn()
            emit_w2t()
            for gp in range(NPAIR):
                astgs = attn_stages(gp, PR_cur)
                if gp + 1 < NPAIR:
                    PR_cur, pstg_next = make_proj(gp + 1)
                    if do_weave:
                        weave(astgs, pstg_next)
                    else:
                        for a in astgs:
                            a()
                        for fn in pstg_next:
                            fn()
                else:
                    weave(astgs, [])

    nc.finalize()
    return nc



_CACHE = {}


def _get_nc(key):
    if key not in _CACHE:
        _CACHE[key] = _build(*key)
    return _CACHE[key]


def kernel(**inputs):
    shared, xts = _prep(inputs)
    skip_kv2 = bool(np.all(shared["kv2b"] == 0.0))
    skip_lnog = bool(np.all(shared["lnogb"] == 1.0))
    skip_lnob = bool(np.all(shared["lnobb"] == 0.0))
    key = (COMPUTE_DT, skip_kv2, skip_lnog, skip_lnob)
    nc = _get_nc(key)

    np_dt = _np_dt(COMPUTE_DT)
    in_maps = []
    for ci in range(N_CORES):
        m = {}
        for name in _SHAPES:
            arr = xts[ci][name] if name in xts[ci] else shared[name]
            pdt = np_dt if name in _DT_PARAMS else np.dtype(np.float32)
            m[name] = np.ascontiguousarray(arr.astype(pdt))
        in_maps.append(m)

    trace = bool(int(os.environ.get("KERNEL_TRACE", "0")))
    # the axon-tunneled device occasionally reports a transient
    # NRT_EXEC_UNIT_UNRECOVERABLE on the first execution after a prior
    # session; a fresh attempt reliably succeeds, so retry a few times
    last_err = None
    for attempt in range(4):
        try:
            res = run_bass_kernel_spmd(nc, in_maps, list(range(N_CORES)),
                                       trace=trace)
            break
        except Exception as e:  # noqa: BLE001
            last_err = e
            import time
            time.sleep(2.0 * (attempt + 1))
    else:
        raise last_err
    kernel.last_results = res
    out = np.concatenate([res.results[ci]["out"] for ci in range(N_CORES)], axis=1)
    return out.astype(np.float32)



# revision 31
# speedup vs baseline: 1.6567x; 1.0294x over previous
"""Trainium2 Bass kernel for nn_ANPToolEncoder (sparse attention encoder).

Sharding: M=64 context groups split across 8 NeuronCores (8 groups each);
the whole network is embarrassingly parallel in M — each core computes
out[:, m_shard, :] and the host concatenates. No collectives.

Layout convention on-chip: activations are kept "feat-major" ([feature
partitions, token free]) so every matmul contraction runs over the
partition axis; softmax denominators for self-attention are computed with
ones-vector matmuls on the PE; the cross-attention softmax denominator is
never computed at all (the final LayerNorm is invariant to per-row scale).
LayerNorm-then-linear (context LN -> V projection) is folded into the V
matmul via host-side weight scaling (wv*g) plus per-token istd/mu
correction terms.
"""

import os
import sys
import numpy as np

for _p in ("/opt/trn_rl_repo", "/root/.axon_site/_ro/trn_rl_repo"):
    if os.path.isdir(_p) and _p not in sys.path:
        sys.path.append(_p)

from concourse import bass, bacc, tile, mybir  # noqa: E402
from concourse.bass_utils import run_bass_kernel_spmd  # noqa: E402

# Pin the ACT function-table chooser to the one set that contains every
# function this kernel uses (exp/ln/relu/square/copy/identity). The default
# greedy chooser ping-pongs between exp_and_others and
# natural_log_exp_and_others, paying a ~1.3us table load dozens of times.
_ACT_PIN = "natural_log_exp_and_others"
_orig_get_act_tables = bacc.get_activation_tables


def _pinned_act_tables(arch):
    t = _orig_get_act_tables(arch)
    return {name: (fns if name == _ACT_PIN else set())
            for name, fns in t.items()}


bacc.get_activation_tables = _pinned_act_tables

B, M, C, DX, H, NH = 256, 64, 256, 512, 512, 8
DH = H // NH
EPS = 1e-5
N_CORES = 8
MLOC = M // N_CORES          # 8 context groups per core
KC = H // 128                # 4 feature chunks of 128
CC = C // 128                # 2 token chunks per group
BC = B // 128                # 2 query chunks

F32 = mybir.dt.float32
BF16 = mybir.dt.bfloat16
ACT = mybir.ActivationFunctionType
ALU = mybir.AluOpType

# compute dtype for matmul-feeding SBUF tensors ("f32" or "bf16")
COMPUTE_DT = os.environ.get("KERNEL_DT", "bf16")
# fp8e4m3 + DoubleRow for the precision-insensitive matmuls (mlp1 img
# part, W2T precompute, cross-attn logits); halves their PE row count
USE_FP8 = bool(int(os.environ.get("KERNEL_FP8", "1"))) and COMPUTE_DT == "bf16"


def _np_dt(dt_str):
    if dt_str == "bf16":
        import ml_dtypes
        return np.dtype(ml_dtypes.bfloat16)
    return np.dtype(np.float32)


def _p2scale(amax, target=96.0):
    """Power-of-two scale placing amax near target (exact in fp formats)."""
    import math
    return float(2.0 ** round(math.log2(target / max(float(amax), 1e-30))))


def _q8(x, s):
    import ml_dtypes
    return np.clip(x * s, -240.0, 240.0).astype(ml_dtypes.float8_e4m3)


def _prep(inp):
    """Host-side weight folds + feat-major layouts. Returns (shared_map, per-core key)."""
    f = {k: np.asarray(v, np.float32) for k, v in inp.items()}
    w1 = f["cp_w1"]                                   # [H, DX+2]
    inw, inb = f["in_w"], f["in_b"]
    bq, bk, bv = inb[:H], inb[H:2*H], inb[2*H:]

    def chunkT(w):      # [out, in] -> [128, in/128, out]  (feat-major, k-chunked)
        wT = w.T                                      # [in, out]
        return np.ascontiguousarray(
            wT.reshape(wT.shape[0] // 128, 128, wT.shape[1]).transpose(1, 0, 2))

    def chunkv(v):      # [H] -> [128, KC]
        return np.ascontiguousarray(v.reshape(KC, 128).T)

    sH = 1.0 / np.sqrt(H)
    shared = {
        "w1kT": chunkT(w1[:, :DX]),                   # [128,4,512]
        "w1gT": np.ascontiguousarray(w1[:, DX:DX+2].T),  # [2,512]
        "b1":   chunkv(f["cp_b1"]),
        "scales": None,
        "w2T":  chunkT(f["cp_w2"]),
        "b2":   chunkv(f["cp_b2"]),
        "inwT": chunkT(inw),                          # [128,4,1536]
        "bq8":  chunkv(bq / np.sqrt(DH)),
        "bk":   chunkv(bk),
        "outwT": chunkT(f["out_w"]),
        "outbcol": chunkv(f["out_b"] + f["out_w"] @ bv),      # [128,KC]
        "wvgT": chunkT(f["wv_w"] * f["lnc_g"][None, :]),
        "kgb":  np.broadcast_to(f["wv_w"] @ f["lnc_g"], (128, H)).copy(),
        "kv2b": np.broadcast_to(f["wv_w"] @ f["lnc_b"] + f["wv_b"], (128, H)).copy(),
        # cross-attn K-proj folded away: logits_T = img @ W2', with
        # W2' = (sH*wk^T wq) @ qe^T + (sH*wk^T wq_b) broadcast; wk_b only
        # scales P~ columns, which the final LN's row-scale invariance eats
        "wkqT": chunkT(sH * f["wk_w"].T @ f["wq_w"]),
        "c2":   chunkv(sH * f["wk_w"].T @ f["wq_b"]),
        "lnogb": np.broadcast_to(f["lno_g"], (128, H)).copy(),
        "lnobb": np.broadcast_to(f["lno_b"], (128, H)).copy(),
        # query embed feat-major: [128, 4, 256]
        "qet": np.ascontiguousarray(
            f["query_embed"].T.reshape(KC, 128, B).transpose(1, 0, 2)),
    }

    # fp8 scales (powers of two; folded into weights/biases here and into
    # eviction-scale immediates in the program)
    img, gt, pr = f["ctx_img_feat"], f["ctx_gt"], f["ctx_pred"]
    if USE_FP8:
        s_x = _p2scale(np.abs(img).max())
        s_w1 = _p2scale(np.abs(shared["w1kT"]).max())
        s_wkq = _p2scale(np.abs(shared["wkqT"]).max())
        s_qe = _p2scale(np.abs(shared["qet"]).max())
        # W2' amax (host matmul only to pick the power-of-two scale)
        w2full = (sH * f["wk_w"].T @ f["wq_w"]) @ f["query_embed"].T
        s_w2 = _p2scale(np.abs(w2full).max(), target=64.0)
        shared["scales"] = (1.0 / (s_w1 * s_x),          # mlp1 eviction
                            s_w2 / (s_wkq * s_qe),       # W2T eviction
                            1.0 / (s_x * s_w2))          # logits exp
        shared["w1kT"] = _q8(shared["w1kT"], s_w1)
        shared["w1gT"] = shared["w1gT"] * (s_w1 * s_x)
        shared["wkqT"] = _q8(shared["wkqT"], s_wkq)
        shared["qet"] = _q8(shared["qet"], s_qe)
        shared["c2"] = shared["c2"] * s_w2

    # per-core X feat-major in k-chunk pairs (DoubleRow rhs layout
    # [128, 2, MLOC*C]) plus xtg [2, MLOC*C] rows gt/pred
    xts = []
    for ci in range(N_CORES):
        gs = slice(ci * MLOC, (ci + 1) * MLOC)
        xi = img[gs].reshape(MLOC * C, DX).T          # [512, 2048]
        xk = xi.reshape(4, 128, MLOC * C).transpose(1, 0, 2)
        if USE_FP8:
            xk = _q8(xk, s_x)
        m = {"xt01": np.ascontiguousarray(xk[:, 0:2, :]),
             "xt23": np.ascontiguousarray(xk[:, 2:4, :]),
             "xtg": np.ascontiguousarray(
                 np.stack([gt[gs].reshape(-1), pr[gs].reshape(-1)]))}
        xts.append(m)
    return shared, xts


# names of DT-typed (matmul-feeding) params; everything else stays f32
_DT_PARAMS = {"w1kT", "w1gT", "w2T", "inwT", "outwT", "wvgT",
              "kgb", "kv2b", "wkqT", "lnogb", "lnobb", "qet",
              "xt01", "xt23", "xtg"}
# params sent as fp8e4m3 when USE_FP8 (pre-scaled host-side)
_FP8_PARAMS = {"w1kT", "wkqT", "qet", "xt01", "xt23"}

_SHAPES = {
    "xt01":  [128, 2, MLOC * C], "xt23": [128, 2, MLOC * C],
    "xtg":   [2, MLOC * C],
    "qet":   [128, KC, B],
    "w1kT":  [128, KC, H], "w1gT": [2, H], "b1": [128, KC],
    "w2T":   [128, KC, H], "b2": [128, KC],
    "inwT":  [128, KC, 3 * H], "bq8": [128, KC], "bk": [128, KC],
    "outwT": [128, KC, H], "outbcol": [128, KC],
    "wvgT":  [128, KC, H], "kgb": [128, H], "kv2b": [128, H],
    "wkqT":  [128, KC, H], "c2": [128, KC],
    "lnogb": [128, H], "lnobb": [128, H],
}

# output dtype: bf16 halves the 4MB/core writeback (host casts back)
OUT_DT = os.environ.get("KERNEL_OUTDT", "bf16")


def _build(dt_str, skip_kv2, skip_lnog, skip_lnob, scales=None):
    DT = BF16 if dt_str == "bf16" else F32
    FP8 = mybir.dt.float8e4
    DR = mybir.MatmulPerfMode.DoubleRow
    fp8 = scales is not None
    nc = bacc.Bacc("TRN2", target_bir_lowering=False, debug=False,
                   num_devices=N_CORES)

    def param_dt(name):
        if fp8 and name in _FP8_PARAMS:
            return FP8
        return DT if name in _DT_PARAMS else F32

    P = {}
    for name, shp in _SHAPES.items():
        P[name] = nc.declare_dram_parameter(name, shp, param_dt(name),
                                            isOutput=False)
    ODT = BF16 if OUT_DT == "bf16" else F32
    out_ext = nc.declare_dram_parameter("out", [B, MLOC, H], ODT, isOutput=True)

    with tile.TileContext(nc) as tc:
        with tc.tile_pool(name="wt", bufs=1) as wt, \
             tc.tile_pool(name="wk", bufs=2) as wk, \
             tc.tile_pool(name="sm", bufs=3) as sm, \
             tc.tile_pool(name="ps", bufs=4, space="PSUM") as psp, \
             tc.tile_pool(name="sc", bufs=2, space="PSUM") as scp:

            # ---- load weights / consts ----
            unused = set()
            if skip_kv2:
                unused.add("kv2b")
            if skip_lnog:
                unused.add("lnogb")
            if skip_lnob:
                unused.add("lnobb")
            # spread input DMAs over the three DMA-capable queues, ordered
            # by first compute use so mlp1/mlp2/qk of pair 0 and the W2T
            # precompute are never starved; scalar kept light (ACT is busy)
            dma_plan = [
                (nc.scalar, ["w1kT", "b1", "w1gT", "w2T", "b2", "bq8", "bk"]),
                (nc.sync,   ["xt01", "xtg", "qet", "wkqT", "c2",
                             "outwT", "outbcol", "wvgT", "kgb",
                             "kv2b", "lnogb", "lnobb"]),
                (nc.gpsimd, ["xt23", "inwT"]),
            ]
            W = {}
            for eng, names in dma_plan:
                for name in names:
                    if name in unused:
                        continue
                    t = wt.tile(_SHAPES[name], param_dt(name), tag=name)
                    if name == "inwT":
                        # q+k halves (first use: p_qk) before the v third
                        eng.dma_start(out=t[:, :, 0:2*H], in_=P[name][:, :, 0:2*H])
                        eng.dma_start(out=t[:, :, 2*H:], in_=P[name][:, :, 2*H:])
                    else:
                        eng.dma_start(out=t[...], in_=P[name][...])
                    W[name] = t
            ones128 = wt.tile([128, 1], DT, tag="ones128")
            nc.vector.memset(ones128[:], 1.0)
            # all-ones stationary: a den matmul with this lhsT yields the
            # key-sum already replicated on every output partition
            ones2d = wt.tile([128, 128], DT, tag="ones2d")
            nc.vector.memset(ones2d[:], 1.0)
            epsc = wt.tile([128, 1], F32, tag="epsc")
            nc.vector.memset(epsc[:], EPS)

            # p-state warmup: dead matmuls keep the PE busy while the first
            # weight DMAs stream in, so real work starts at a higher clock
            warm = psp.tile([128, 128], F32, tag="ps", name="warm")
            for i in range(12):
                nc.tensor.matmul(warm[...], ones2d[...], ones2d[...],
                                 start=(i == 0), stop=(i == 11),
                                 skip_group_check=True)

            def mm_chain(ps_ap, pairs):
                """Accumulating matmul chain: pairs = [(lhsT, rhs), ...]."""
                n = len(pairs)
                for i, (l, r) in enumerate(pairs):
                    nc.tensor.matmul(ps_ap, l, r, start=(i == 0), stop=(i == n - 1),
                                     skip_group_check=True)

            # ---- W2' = WKQ @ qe^T + c2 (once; replaces Q and K projs).
            # Emitted AFTER pair-0 projections: the in-order PE stream must
            # not start with matmuls that wait on late-arriving DMAs. ----
            W2T = wt.tile([128, KC, B], FP8 if fp8 else DT, tag="W2T")

            def emit_w2t():
                for dxc in range(KC):
                    ps = psp.tile([128, B], F32, tag="ps")
                    if fp8:
                        for c in range(2):
                            nc.tensor.matmul(
                                ps[...],
                                W["wkqT"][:, 2*c:2*c+2, dxc*128:(dxc+1)*128],
                                W["qet"][:, 2*c:2*c+2, :],
                                start=(c == 0), stop=(c == 1), perf_mode=DR,
                                skip_group_check=True)
                        nc.scalar.activation(W2T[:, dxc, :], ps[...],
                                             ACT.Identity, scale=scales[1],
                                             bias=W["c2"][:, dxc:dxc+1])
                    else:
                        mm_chain(ps[...],
                                 [(W["wkqT"][:, k, dxc*128:(dxc+1)*128],
                                   W["qet"][:, k, :]) for k in range(KC)])
                        nc.scalar.activation(W2T[:, dxc, :], ps[...],
                                             ACT.Identity,
                                             bias=W["c2"][:, dxc:dxc+1])

            # ---- software-pipelined group-pair schedule ----
            # Projections for pair gp+1 are woven between the attention
            # stages of pair gp so the PE stream stays dense (HAM warm).

            def make_proj(gp):
                """Emitters for pair gp's batched (N=512) projections."""
                xg2 = slice(gp * 2 * C, (gp + 1) * 2 * C)
                PR = {}

                def p_mlp1():
                    h1 = wk.tile([128, KC, 2 * C], DT, tag="h1")
                    for hc in range(KC):
                        hsl = slice(hc * 128, (hc + 1) * 128)
                        ps = psp.tile([128, 2 * C], F32, tag="ps")
                        if fp8:
                            for c, xt8 in enumerate((W["xt01"], W["xt23"])):
                                nc.tensor.matmul(
                                    ps[...], W["w1kT"][:, 2*c:2*c+2, hsl],
                                    xt8[:, :, xg2], start=(c == 0), stop=False,
                                    perf_mode=DR, skip_group_check=True)
                        else:
                            for k in range(4):
                                xt8 = W["xt01"] if k < 2 else W["xt23"]
                                nc.tensor.matmul(
                                    ps[...], W["w1kT"][:, k, hsl],
                                    xt8[:, k % 2, xg2], start=(k == 0),
                                    stop=False, skip_group_check=True)
                        nc.tensor.matmul(ps[...], W["w1gT"][0:2, hsl],
                                         W["xtg"][0:2, xg2], start=False,
                                         stop=True, skip_group_check=True)
                        nc.scalar.activation(
                            h1[:, hc, :], ps[...], ACT.Relu,
                            scale=(scales[0] if fp8 else 1.0),
                            bias=W["b1"][:, hc:hc+1])
                    PR["h1"] = h1

                def p_mlp2():
                    h1 = PR["h1"]
                    ctx = wk.tile([128, KC, 2 * C], DT, tag="ctx")
                    for hc in range(KC):
                        ps = psp.tile([128, 2 * C], F32, tag="ps")
                        mm_chain(ps[...], [(W["w2T"][:, k, hc*128:(hc+1)*128],
                                            h1[:, k, :]) for k in range(KC)])
                        nc.vector.tensor_scalar(ctx[:, hc, :], ps[...],
                                                W["b2"][:, hc:hc+1], None, ALU.add)
                    PR["ctx"] = ctx

                def p_qk():
                    ctx = PR["ctx"]
                    qk = wk.tile([128, 2 * KC, 2 * C], DT, tag="qk")
                    for jc in range(2 * KC):
                        ps = psp.tile([128, 2 * C], F32, tag="ps")
                        mm_chain(ps[...], [(W["inwT"][:, k, jc*128:(jc+1)*128],
                                            ctx[:, k, :]) for k in range(KC)])
                        if jc < KC:
                            nc.vector.tensor_scalar(qk[:, jc, :], ps[...],
                                                    1.0 / float(np.sqrt(DH)),
                                                    W["bq8"][:, jc:jc+1],
                                                    ALU.mult, ALU.add)
                        else:
                            nc.vector.tensor_scalar(qk[:, jc, :], ps[...],
                                                    W["bk"][:, jc-KC:jc-KC+1], None,
                                                    ALU.add)
                    PR["qk"] = qk

                def p_v():
                    ctx = PR["ctx"]
                    vtok = wk.tile([128, 2 * CC, H], DT, tag="vtok")
                    for cc2 in range(2 * CC):
                        ps = psp.tile([128, H], F32, tag="ps")
                        mm_chain(ps[...], [(ctx[:, k, cc2*128:(cc2+1)*128],
                                            W["inwT"][:, k, 2*H:3*H])
                                           for k in range(KC)])
                        nc.scalar.activation(vtok[:, cc2, :], ps[...], ACT.Copy)
                    PR["vtok"] = vtok

                return PR, [p_mlp1, p_mlp2, p_qk, p_v]

            pmode = int(os.environ.get('KERNEL_PAIRS', '2'))
            if pmode == 2:
                # opposite-half pairs: each head's scores go to its OWN psum
                # tile, so the PE overlaps head B's LDWEIGHTS with head A's
                # matmul (different row groups, different banks - safe)
                PAIRS = [(0, 1), (2, 3), (4, 5), (6, 7)]
            elif pmode == 1:
                PAIRS = [(0, 2), (1, 3), (4, 6), (5, 7)]
            else:
                PAIRS = [(0, 2), (4, 6), (1, 3), (5, 7)]

            def attn_stages(gp, PR):
                """Per-group attention stage emitters for pair gp, already
                interleaved over the pair's two groups."""
                qk, vtok, ctx = PR["qk"], PR["vtok"], PR["ctx"]
                S = {0: {}, 1: {}}

                def s1_scores(g2):
                    cg = slice(g2 * C, (g2 + 1) * C)
                    PTs = []
                    for p, pair in enumerate(PAIRS):
                        PT = wk.tile([128, CC, 2 * C], DT, tag=f"PT{p}")
                        PTs.append(PT)
                        for kc in range(CC):
                            # one 2-bank tile: each bank written by one PE
                            # row group (hazard-safe); both heads evicted
                            # by a single strided exp
                            sc2 = scp.tile([128, 2, 512], F32, tag="sc2")
                            for hh, h in enumerate(pair):
                                off = 64 * (h % 2)
                                jslot = h // 2
                                lhsT = qk[off:off+64, KC + jslot,
                                          g2*C + kc*128: g2*C + (kc+1)*128]
                                rhs = qk[off:off+64, jslot, cg]
                                nc.tensor.matmul(sc2[:, hh, 0:C], lhsT, rhs,
                                                 start=True, stop=True,
                                                 skip_group_check=True)
                            nc.scalar.activation(PT[:, kc, :],
                                                 sc2[:, :, 0:C], ACT.Exp)
                    S[g2]["PTs"] = PTs
                    # denominators after all exps are in flight: the all-ones
                    # stationary replicates each key-sum to every partition,
                    # so no separate broadcast step is needed
                    invs = []
                    for p in range(len(PAIRS)):
                        den_ps = psp.tile([128, 2 * C], F32, tag="ps",
                                          name=f"den{p}")
                        for kc in range(CC):
                            nc.tensor.matmul(den_ps[...], ones2d[...],
                                             PTs[p][:, kc, :],
                                             start=(kc == 0), stop=(kc == CC - 1),
                                             skip_group_check=True)
                        inv_bc = wk.tile([128, 2 * C], F32, tag=f"inv{p}")
                        nc.vector.reciprocal_approx_fast(inv_bc[...], den_ps[...])
                        invs.append(inv_bc)
                    S[g2]["invs"] = invs

                def s3_sa(g2):
                    invs = S[g2]["invs"]
                    saT = wk.tile([128, KC, C], DT, tag="saT")
                    for p, pair in enumerate(PAIRS):
                        PT = S[g2]["PTs"][p]
                        sa0 = psp.tile([128, C], F32, tag="ps")
                        for hh, h in enumerate(pair):
                            mm_chain(sa0[64*hh:64*hh+64, :],
                                     [(vtok[:, 2*g2 + kc, 64*h:64*h+64],
                                       PT[:, kc, hh*C:(hh+1)*C])
                                      for kc in range(CC)])
                        for hh, h in enumerate(pair):
                            o = 64 * (h % 2)
                            nc.vector.tensor_tensor(saT[o:o+64, h // 2, :],
                                                    sa0[64*hh:64*hh+64, :],
                                                    invs[p][o:o+64,
                                                            hh*C:(hh+1)*C],
                                                    ALU.mult)
                    S[g2]["saT"] = saT

                def s4_outproj(g2):
                    cg = slice(g2 * C, (g2 + 1) * C)
                    saT = S[g2]["saT"]
                    rT = wk.tile([128, KC, C], DT, tag="rT")
                    r2T = wk.tile([128, KC, C], DT, tag="r2T")
                    for hc in range(KC):
                        ps = psp.tile([128, C], F32, tag="ps")
                        mm_chain(ps[...], [(W["outwT"][:, k, hc*128:(hc+1)*128],
                                            saT[:, k, :]) for k in range(KC)])
                        # bias folded in as a per-partition scalar: saves the
                        # K=1 bias-row matmul on the PE
                        nc.vector.scalar_tensor_tensor(
                            rT[:, hc, :], ps[...], W["outbcol"][:, hc:hc+1],
                            ctx[:, hc, cg], ALU.add, ALU.add)
                        nc.scalar.activation(r2T[:, hc, :], rT[:, hc, :],
                                             ACT.Square)
                    S[g2]["rT"] = rT
                    S[g2]["r2T"] = r2T

                def s5_stats(g2):
                    # both token chunks' LN stats in [128,2]-wide ops; the
                    # sign of t = mu*istd is folded into s6's subtract
                    rT, r2T = S[g2]["rT"], S[g2]["r2T"]
                    stat = psp.tile([128, 4], F32, tag="ps", name="stat")
                    for cc in range(CC):
                        mm_chain(stat[:, cc:cc+1],
                                 [(rT[:, k, cc*128:(cc+1)*128], ones128[:, 0:1])
                                  for k in range(KC)])
                        mm_chain(stat[:, 2+cc:3+cc],
                                 [(r2T[:, k, cc*128:(cc+1)*128], ones128[:, 0:1])
                                  for k in range(KC)])
                    mu = sm.tile([128, 2], F32, tag="mu")
                    nc.vector.tensor_scalar(mu[...], stat[:, 0:2], 1.0 / H, None,
                                            ALU.mult)
                    s1t = sm.tile([128, 2], F32, tag="s1")
                    nc.vector.tensor_scalar(s1t[...], stat[:, 2:4], 1.0 / H, EPS,
                                            ALU.mult, ALU.add)
                    musq = sm.tile([128, 2], F32, tag="musq")
                    nc.scalar.activation(musq[...], mu[...], ACT.Square)
                    vpe = sm.tile([128, 2], F32, tag="vpe")
                    nc.vector.tensor_tensor(vpe[...], s1t[...], musq[...],
                                            ALU.subtract)
                    lnv = sm.tile([128, 2], F32, tag="lnv")
                    nc.scalar.activation(lnv[...], vpe[...], ACT.Ln)
                    istd = sm.tile([128, 2], F32, tag="istd")
                    nc.scalar.activation(istd[...], lnv[...], ACT.Exp, scale=-0.5)
                    t_ = sm.tile([128, 2], F32, tag="t_")
                    nc.vector.tensor_tensor(t_[...], mu[...], istd[...], ALU.mult)
                    S[g2]["istds"] = [istd[:, cc:cc+1] for cc in range(CC)]
                    S[g2]["ts"] = [t_[:, cc:cc+1] for cc in range(CC)]

                def s6_v(g2):
                    rT = S[g2]["rT"]
                    V = wk.tile([128, CC, H], DT, tag="V")
                    for cc in range(CC):
                        ps = psp.tile([128, H], F32, tag="ps")
                        mm_chain(ps[...], [(rT[:, k, cc*128:(cc+1)*128],
                                            W["wvgT"][:, k, :]) for k in range(KC)])
                        tmp1 = sm.tile([128, H], DT, tag="tmp1")
                        nc.scalar.activation(tmp1[...], ps[...], ACT.Copy,
                                             scale=S[g2]["istds"][cc][...])
                        tmp2 = sm.tile([128, H], DT, tag="tmp2")
                        nc.vector.tensor_scalar(tmp2[...], W["kgb"][...],
                                                S[g2]["ts"][cc][...], None,
                                                ALU.mult)
                        if skip_kv2:
                            nc.vector.tensor_tensor(V[:, cc, :], tmp1[...],
                                                    tmp2[...], ALU.subtract)
                        else:
                            tmp3 = sm.tile([128, H], DT, tag="tmp3")
                            nc.vector.tensor_tensor(tmp3[...], tmp1[...],
                                                    tmp2[...], ALU.subtract)
                            nc.vector.tensor_tensor(V[:, cc, :], tmp3[...],
                                                    W["kv2b"][...], ALU.add)
                    S[g2]["V"] = V

                def s7_logits(g2):
                    g = gp * 2 + g2
                    PTc = wk.tile([128, CC, B], DT, tag="PTc")
                    for cc in range(CC):
                        gsl = slice(g*C + cc*128, g*C + (cc+1)*128)
                        ps = psp.tile([128, B], F32, tag="ps")
                        if fp8:
                            for c, xt8 in enumerate((W["xt01"], W["xt23"])):
                                nc.tensor.matmul(
                                    ps[...], xt8[:, :, gsl],
                                    W2T[:, 2*c:2*c+2, :],
                                    start=(c == 0), stop=(c == 1),
                                    perf_mode=DR, skip_group_check=True)
                            nc.scalar.activation(PTc[:, cc, :], ps[...],
                                                 ACT.Exp, scale=scales[2])
                        else:
                            mm_chain(ps[...],
                                     [((W["xt01"] if kx < 2 else
                                        W["xt23"])[:, kx % 2, gsl],
                                       W2T[:, kx, :]) for kx in range(4)])
                            nc.scalar.activation(PTc[:, cc, :], ps[...], ACT.Exp)
                    S[g2]["PTc"] = PTc

                def s8_out(g2):
                    g = gp * 2 + g2
                    out_engs = [nc.sync, nc.gpsimd, nc.scalar]
                    PTc, V = S[g2]["PTc"], S[g2]["V"]
                    for bc2 in range(BC):
                        z0 = psp.tile([128, H], F32, tag="ps")
                        mm_chain(z0[...], [(PTc[:, kc, bc2*128:(bc2+1)*128],
                                            V[:, kc, :]) for kc in range(CC)])
                        bns = sm.tile([128, 6], F32, tag="bns")
                        nc.vector.bn_stats(bns[...], z0[...])
                        ms = sm.tile([128, 2], F32, tag="ms")
                        nc.vector.bn_aggr(ms[...], bns[...])
                        lnv = sm.tile([128, 1], F32, tag="lnvz")
                        nc.scalar.activation(lnv[...], ms[:, 1:2], ACT.Ln,
                                             bias=epsc[...])
                        istd = sm.tile([128, 1], F32, tag="istdz")
                        nc.scalar.activation(istd[...], lnv[...], ACT.Exp,
                                             scale=-0.5)
                        nmi = sm.tile([128, 1], F32, tag="nmi")
                        nc.vector.tensor_scalar(nmi[...], ms[:, 0:1], istd[...],
                                                -1.0, ALU.mult, ALU.mult)
                        if skip_lnog and skip_lnob:
                            o_sb = sm.tile([128, H], ODT, tag="osb")
                            nc.scalar.activation(o_sb[...], z0[...], ACT.Identity,
                                                 scale=istd[...], bias=nmi[...])
                        else:
                            t1 = sm.tile([128, H], F32, tag="t1")
                            nc.scalar.activation(t1[...], z0[...], ACT.Identity,
                                                 scale=istd[...], bias=nmi[...])
                            o_sb = sm.tile([128, H], ODT, tag="osb")
                            if skip_lnog:
                                nc.vector.tensor_tensor(o_sb[...], t1[...],
                                                        W["lnobb"][...], ALU.add)
                            elif skip_lnob:
                                nc.vector.tensor_tensor(o_sb[...], t1[...],
                                                        W["lnogb"][...], ALU.mult)
                            else:
                                t2 = sm.tile([128, H], F32, tag="t2")
                                nc.vector.tensor_tensor(t2[...], t1[...],
                                                        W["lnogb"][...], ALU.mult)
                                nc.vector.tensor_tensor(o_sb[...], t2[...],
                                                        W["lnobb"][...], ALU.add)
                        eng = out_engs[(g * BC + bc2) % len(out_engs)]
                        eng.dma_start(out=out_ext[bc2*128:(bc2+1)*128, g, :],
                                      in_=o_sb[...])

                out = []
                for stg in (s1_scores, s3_sa, s4_outproj, s5_stats,
                            s6_v, s7_logits, s8_out):
                    out.append(lambda stg=stg: stg(0))
                    out.append(lambda stg=stg: stg(1))
                return out

            def weave(astgs, pstgs):
                """Emit attention chunks with proj stages spread between them."""
                if not pstgs:
                    for a in astgs:
                        a()
                    return
                # insert a proj stage after every ceil(len/|p|) attention chunks
                k = max(1, len(astgs) // (len(pstgs) + 1))
                pi = 0
                for i, a in enumerate(astgs):
                    a()
                    if (i + 1) % k == 0 and pi < len(pstgs):
                        pstgs[pi]()
                        pi += 1
                while pi < len(pstgs):
                    pstgs[pi]()
                    pi += 1

            NPAIR = MLOC // 2
            do_weave = int(os.environ.get('KERNEL_WEAVE', '0'))
            PR_cur, pstg_cur = make_proj(0)
            for fn in pstg_cur:
                fn()
            emit_w2t()
            for gp in range(NPAIR):
                astgs = attn_stages(gp, PR_cur)
                if gp + 1 < NPAIR:
                    PR_cur, pstg_next = make_proj(gp + 1)
                    if do_weave:
                        weave(astgs, pstg_next)
                    else:
                        for a in astgs:
                            a()
                        for fn in pstg_next:
                            fn()
                else:
                    weave(astgs, [])

    nc.finalize()
    return nc



_CACHE = {}


def _get_nc(key):
    if key not in _CACHE:
        _CACHE[key] = _build(*key[:4], scales=key[4])
    return _CACHE[key]


def kernel(**inputs):
    shared, xts = _prep(inputs)
    skip_kv2 = bool(np.all(shared["kv2b"] == 0.0))
    skip_lnog = bool(np.all(shared["lnogb"] == 1.0))
    skip_lnob = bool(np.all(shared["lnobb"] == 0.0))
    scales = shared["scales"]
    key = (COMPUTE_DT, skip_kv2, skip_lnog, skip_lnob, scales)
    nc = _get_nc(key)

    np_dt = _np_dt(COMPUTE_DT)
    fp8_names = _FP8_PARAMS if scales is not None else set()
    in_maps = []
    for ci in range(N_CORES):
        m = {}
        for name in _SHAPES:
            arr = xts[ci][name] if name in xts[ci] else shared[name]
            if name in fp8_names:
                pdt = arr.dtype       # already fp8 from _prep
            elif name in _DT_PARAMS:
                pdt = np_dt
            else:
                pdt = np.dtype(np.float32)
            m[name] = np.ascontiguousarray(arr.astype(pdt))
        in_maps.append(m)

    trace = bool(int(os.environ.get("KERNEL_TRACE", "0")))
    # the axon-tunneled device occasionally reports a transient
    # NRT_EXEC_UNIT_UNRECOVERABLE on the first execution after a prior
    # session; a fresh attempt reliably succeeds, so retry a few times
    last_err = None
    for attempt in range(4):
        try:
            res = run_bass_kernel_spmd(nc, in_maps, list(range(N_CORES)),
                                       trace=trace)
            break
        except Exception as e:  # noqa: BLE001
            last_err = e
            import time
            time.sleep(2.0 * (attempt + 1))
    else:
        raise last_err
    kernel.last_results = res
    out = np.concatenate([res.results[ci]["out"] for ci in range(N_CORES)], axis=1)
    return out.astype(np.float32)

